# revision 30
# baseline (speedup 1.0000x reference)
"""Distributed Trainium2 (Bass/Tile) kernel for single-head latent attention.

Reference computation (B=4, S=4096, D=1024, DL=64):
    qkv = x @ Wd + bd; q,k,v = split(qkv)
    logits = (q @ k^T) / sqrt(DL) / TEMP, key-masked
    out = softmax(logits) @ v @ Wu + bu

Sharding: data-parallel over (batch, seq-half) -> 8 shards of 2048 query rows.
Each core recomputes K/V for its batch's keys from x (no collectives).

v2 design (vs the 124us baseline):
  - Fine-grained range-major input DMA on 3 queues; projections chase the
    arriving data and attention pair 0 starts at ~8us instead of ~35us.
  - MM1 row-tiled (contraction DL=64): chunk pairs run concurrently on the
    two 64-row halves of the PE array (tile_position (0,0)/(64,0)).  kT uses
    a parity layout (even kv-ranges' k in partitions 0:64, odd ranges' in
    64:128) produced with per-range-swapped [k|v]/[v|k] projection weights,
    so no cross-partition moves are needed anywhere.  q is duplicated into
    both halves for free by duplicating Wd's q columns (M=64 -> M=128).
  - Postponed normalization: out = (ctxU @ Wu) * (1/Z) + bu.  1/Z is applied
    per-query at PSUM evacuation (scalar_tensor_tensor), ctxU flows bf16
    (unnormalized ctx spans e^+-55 -> overflows fp16 but not bf16).  Z is
    transposed to query-major via 16 tiny PE transposes + one reciprocal.
  - Scalar engine does exp ONLY (34 x [128,1024] ACTIVATEs ~ 34us is the
    serial floor); every copy/evac lives on vector/gpsimd.
  - Up-projection row-tiled too: ctxU duplicated into partitions 64:128 by
    an SBUF->SBUF DMA; odd st tiles run on T8 with Wu's duplicate rows.
  - Dummy matmuls only bridge the initial DMA ramp (HAM clock warm-up).
"""

import sys

if "/opt/trn_rl_repo" not in sys.path:
    sys.path.insert(0, "/opt/trn_rl_repo")

import numpy as np

from concourse import bacc, tile
from concourse import mybir
from concourse.masks import make_identity

F32 = mybir.dt.float32
F32R = mybir.dt.float32r
BF16 = mybir.dt.bfloat16
F16 = mybir.dt.float16

B, S, D, DL = 4, 4096, 1024, 64
N_CORES = 8
S_LOC = S // 2          # 2048 query rows per core
SR = 512
JC = 128                # key chunk
NJK = 17                # compacted key chunks
K_CAP = NJK * JC        # 2176 >= max unmasked keys per batch
QH = 1024               # one attention pass = 1024 query columns
SCALE = 1.25            # 1/sqrt(64)/0.1
LOGIT_SHIFT = -40.0
MASKED_BIAS = -1e30
UP_PACKED = True        # odd st tiles on T8 (needs ctxU dup DMA)

# kv ranges: (col0, width, parity).  Even ranges project with [k|v] weights
# (k -> psum rows 0:64), odd with [v|k] (k -> rows 64:128), so the k-half
# always evacuates same-partition into its kT2 half.
KV_RANGES = [(0, 512, 0), (512, 512, 1), (1024, 512, 0), (1536, 512, 1),
             (2048, 128, 0)]
# chunk -> kT2 half/block: top (rows 0:64) = ranges 0,2,4; bottom = 1,3
TOP_CHUNKS = [0, 1, 2, 3, 8, 9, 10, 11, 16]
BOT_CHUNKS = [4, 5, 6, 7, 12, 13, 14, 15]
# step schedule ordered by DMA arrival (r0 first, r3 last); top/bot pairs
# run concurrently on the two array halves, same-half pairs just serialize
STEPS = [(0, 1), (2, 4), (3, 5), (6, 7), (8, 9), (10, 12), (11, 13),
         (14, 15), (16, None)]
SOLO = 16

_CACHE = {}


def _chunk_block(c):
    """kT2 (half, block) for chunk c."""
    if c in TOP_CHUNKS:
        return 0, TOP_CHUNKS.index(c)
    return 1, BOT_CHUNKS.index(c)


def build_graph():
    nc = bacc.Bacc("TRN2", target_bir_lowering=False, debug=False,
                   num_devices=N_CORES)

    # partition-major layouts: one big DMA can carry a (slab-range, col-range)
    xT_d = nc.dram_tensor("xT", [128, 8, S_LOC], F16, kind="ExternalInput").ap()
    xk_d = nc.dram_tensor("xkT", [128, 8, K_CAP], F16, kind="ExternalInput").ap()
    wd_d = nc.dram_tensor("Wd2", [128, 8 * 384], F16, kind="ExternalInput").ap()
    wu_d = nc.dram_tensor("Wu2", [128, D], BF16, kind="ExternalInput").ap()
    bu_d = nc.dram_tensor("bu_row", [1, D], F32, kind="ExternalInput").ap()
    bdq_d = nc.dram_tensor("bd_q2", [128, 1], F32, kind="ExternalInput").ap()
    bdkv_d = nc.dram_tensor("bd_kv2", [128, 2], F32, kind="ExternalInput").ap()
    mb_d = nc.dram_tensor("maskbias", [128, NJK], F32, kind="ExternalInput").ap()
    out_d = nc.dram_tensor("out", [S_LOC, D], F16, kind="ExternalOutput").ap()

    with tile.TileContext(nc) as tc, nc.allow_low_precision(
            reason="bf16/f16 tiles feed full-rate PE matmuls; ~10-bit "
                   "mantissas are far inside the 2e-2 error budget"):
        with (
            tc.tile_pool(name="consts", bufs=1) as consts,
            tc.tile_pool(name="acts", bufs=1) as acts,
            tc.tile_pool(name="ep", bufs=4) as ep,
            tc.tile_pool(name="ob", bufs=3) as ob,
            tc.tile_pool(name="PL", bufs=2, space="PSUM") as PL,
            tc.tile_pool(name="PC", bufs=1, space="PSUM") as PC,
            tc.tile_pool(name="PP", bufs=1, space="PSUM") as PP,
            tc.tile_pool(name="PT", bufs=1, space="PSUM") as PT,
        ):
            # ---- tiny consts built on-device (no DMA dependency) ----------
            seed = consts.tile([128, SR], F16)
            nc.vector.memset(seed[:], 0.25)
            # exp ACT table preload (~2.7us) during the DMA ramp
            actwarm = consts.tile([128, 32], F32)
            nc.scalar.activation(actwarm[:], seed[:, 0:32],
                                 mybir.ActivationFunctionType.Exp)
            identf = consts.tile([128, 64], F32)
            nc.vector.memset(identf[:], 0.0)
            make_identity(nc, identf[0:64, :], nomemset=True)
            make_identity(nc, identf[64:128, :], nomemset=True)
            ident = consts.tile([128, 64], F32R)
            nc.vector.tensor_copy(ident[:], identf[:])
            onesf = consts.tile([128, 4], F32)
            nc.vector.memset(onesf[:], 1.0)
            onesr = consts.tile([128, 4], F32R)
            nc.vector.tensor_copy(onesr[:], onesf[:])
            onesrow = consts.tile([1, 128], F32)
            nc.vector.memset(onesrow[:], 1.0)

            # ---- DMA'd consts (gpsimd queue, small-first) -----------------
            bdq_s = consts.tile([128, 1], F32)
            nc.gpsimd.dma_start(out=bdq_s[:], in_=bdq_d[:])
            bdkv_s = consts.tile([128, 2], F32)
            nc.gpsimd.dma_start(out=bdkv_s[:], in_=bdkv_d[:])
            mb_s = consts.tile([128, NJK], F32)
            nc.gpsimd.dma_start(out=mb_s[:], in_=mb_d[:])
            wd_s = consts.tile([128, 8 * 384], F16)   # DMA'd below (fast q)
            wu_s = consts.tile([128, D], BF16)
            bur_s = consts.tile([1, D], F32)

            # ---- activation tiles -----------------------------------------
            xq_sb = acts.tile([128, 8, S_LOC], F16)
            xk_sb = acts.tile([128, 8, K_CAP], F16)
            qT2 = acts.tile([128, S_LOC], F16)       # q duplicated both halves
            kT2 = acts.tile([128, 9 * JC], F16)      # parity layout
            vTb = acts.tile([128, K_CAP], F32R)      # v staging (half by range)
            v_aug = acts.tile([128, NJK * 65], BF16)  # [v(64) | ones] per chunk
            nc.vector.memset(v_aug[:], 1.0)
            ctxu = acts.tile([128, S_LOC], BF16)     # rows 0:64 ctx, 64:128 dup
            zr = acts.tile([128, S_LOC], F32)        # Z row staging (row 64)
            rzbc = acts.tile([128, 16], F32)         # 1/Z, query-major, col=st
            bub_s = consts.tile([128, D], F32)       # bu broadcast to 128 rows

            # ---- input DMAs: big slab-group triggers on the two fast HW
            # queues (sync lo-slabs, scalar hi-slabs), priority order.
            # gpsimd's queue is software-DGE (slow) -> tiny/late consts only.
            def xk_dma(eng, k0, k1, c0, c1):
                eng.dma_start(out=xk_sb[:, k0:k1, c0:c1],
                              in_=xk_d[:, k0:k1, c0:c1])

            def xq_dma(eng, k0, k1, c0, c1):
                eng.dma_start(out=xq_sb[:, k0:k1, c0:c1],
                              in_=xT_d[:, k0:k1, c0:c1])

            for (lo, hi), eng in (((0, 4), nc.sync), ((4, 8), nc.scalar)):
                nc_e = eng
                nc_e.dma_start(out=wd_s[:, lo * 384:hi * 384],
                               in_=wd_d[:, lo * 384:hi * 384])
                xk_dma(nc_e, lo, hi, 0, 512)
                xq_dma(nc_e, lo, hi, 0, 1024)
                xk_dma(nc_e, lo, hi, 512, 1024)
                xk_dma(nc_e, lo, hi, 1024, K_CAP)
                xq_dma(nc_e, lo, hi, 1024, 2048)
            # wu/bu only needed from pass 1 (~25us): slow queue is fine
            nc.gpsimd.dma_start(out=wu_s[:], in_=wu_d[:])
            nc.gpsimd.dma_start(out=bur_s[:], in_=bu_d[:])

            # ---- helpers --------------------------------------------------
            ndum = [0]

            def warm(n):
                for _ in range(n):
                    dmy = PL.tile([128, QH], F32, tag="L",
                                  name=f"dmy{ndum[0]}")
                    ndum[0] += 1
                    nc.tensor.matmul(dmy[:, 0:SR], seed[:, 0:128],
                                     seed[:], start=True, stop=True)

            def q_range(r):
                ps_q = PP.tile([128, SR], F32, tag="p", name=f"psq{r}")
                for k in range(8):
                    nc.tensor.matmul(
                        ps_q[:], wd_s[:, k * 384:k * 384 + 128],
                        xq_sb[:, k, r * SR:(r + 1) * SR],
                        start=(k == 0), stop=(k == 7))
                nc.vector.tensor_scalar_add(qT2[:, r * SR:(r + 1) * SR],
                                            ps_q[:], bdq_s[:, 0:1])

            def kv_range(ri):
                c0, w, par = KV_RANGES[ri]
                ps_kv = PP.tile([128, SR], F32, tag="p", name=f"pskv{ri}")
                wcol = 128 if par == 0 else 256
                for k in range(8):
                    nc.tensor.matmul(
                        ps_kv[:, 0:w],
                        wd_s[:, k * 384 + wcol:k * 384 + wcol + 128],
                        xk_sb[:, k, c0:c0 + w],
                        start=(k == 0), stop=(k == 7))
                half, blk0 = _chunk_block(c0 // JC)
                kh = slice(0, 64) if half == 0 else slice(64, 128)
                vh = slice(64, 128) if half == 0 else slice(0, 64)
                nc.vector.tensor_scalar_add(
                    kT2[kh, blk0 * JC:blk0 * JC + w], ps_kv[kh, 0:w],
                    bdkv_s[kh, par:par + 1])
                nc.vector.tensor_scalar_add(
                    vTb[vh, c0:c0 + w], ps_kv[vh, 0:w],
                    bdkv_s[vh, par:par + 1])

            def v_trans(ri):
                c0, w, par = KV_RANGES[ri]
                vh = slice(64, 128) if par == 0 else slice(0, 64)
                idh = ident[64:128, :] if par == 0 else ident[0:64, :]
                nch = w // JC
                vt_ps = PT.tile([128, 256], F32R, tag="t", name=f"vt{ri}")
                for j in range(nch):
                    c = c0 // JC + j
                    nc.tensor.transpose(
                        vt_ps[:, j * 64:(j + 1) * 64],
                        vTb[vh, c * JC:(c + 1) * JC], idh)
                for j in range(nch):
                    c = c0 // JC + j
                    nc.vector.tensor_copy(v_aug[:, c * 65:c * 65 + 64],
                                          vt_ps[:, j * 64:(j + 1) * 64])

            def bu_bcast():
                # bu broadcast: [1,1024] -> [128,1024] via two K=1 matmuls
                for s2 in range(2):
                    bu_ps = PT.tile([128, SR], F32, tag="t", name=f"bups{s2}")
                    nc.tensor.matmul(bu_ps[:], onesrow[:, 0:128],
                                     bur_s[:, s2 * SR:(s2 + 1) * SR],
                                     start=True, stop=True)
                    nc.vector.tensor_copy(bub_s[:, s2 * SR:(s2 + 1) * SR],
                                          bu_ps[:])

            warm(3)

            # ================ main software pipeline =======================
            exs = {}            # chunk -> ex tile (per pass, overwritten)
            ctx_tiles = {}
            nmm2 = [0]

            def mm1_exp(pas, ce, co):
                q0 = pas * QH
                lgs = []
                for c in (ce, co):
                    if c is None:
                        continue
                    half, blk = _chunk_block(c)
                    hs = slice(0, 64) if half == 0 else slice(64, 128)
                    lg = PL.tile([128, QH], F32, tag="L",
                                 name=f"lg{pas}_{c}")
                    for s2 in range(2):
                        nc.tensor.matmul(
                            lg[:, s2 * SR:(s2 + 1) * SR],
                            kT2[hs, blk * JC:(blk + 1) * JC],
                            qT2[hs, q0 + s2 * SR:q0 + (s2 + 1) * SR],
                            start=True, stop=True)
                    lgs.append((c, lg))
                for c, lg in lgs:
                    ex = ep.tile([128, QH], BF16, tag="e", name=f"ex{pas}_{c}")
                    nc.scalar.activation(ex[:], lg[:],
                                         mybir.ActivationFunctionType.Exp,
                                         bias=mb_s[:, c:c + 1], scale=SCALE)
                    exs[c] = ex

            def mm2(pas, c):
                ctx_ps = ctx_tiles[pas]
                i = nmm2[0]
                nmm2[0] += 1
                first = (i % NJK == 0)
                last = (i % NJK == NJK - 1)
                for s2 in range(2):
                    nc.tensor.matmul(
                        ctx_ps[:, s2 * SR:(s2 + 1) * SR],
                        v_aug[:, c * 65:(c + 1) * 65],
                        exs[c][:, s2 * SR:(s2 + 1) * SR],
                        start=first, stop=last)

            def ctx_evac(pas):
                q0 = pas * QH
                ctx_ps = ctx_tiles[pas]
                nc.vector.tensor_copy(ctxu[0:64, q0:q0 + QH], ctx_ps[0:64, :])
                nc.vector.tensor_copy(zr[64:65, q0:q0 + QH], ctx_ps[64:65, :])
                if UP_PACKED:
                    nc.sync.dma_start(out=ctxu[64:128, q0:q0 + QH],
                                      in_=ctxu[0:64, q0:q0 + QH])

            def z_recip(pas):
                # transpose Z [1,1024] -> query-major [128,8] via 8 rank-1
                # matmuls (lhsT = Z block, rhs = scalar 1.0), then reciprocal
                q0 = pas * QH
                zt_ps = PT.tile([128, 16], F32, tag="t", name=f"zt{pas}")
                for st in range(8):
                    nc.tensor.matmul(
                        zt_ps[:, st:st + 1],
                        zr[64:65, q0 + st * JC:q0 + (st + 1) * JC],
                        onesf[64:65, 0:1], start=True, stop=True)
                nc.vector.reciprocal(rzbc[:, pas * 8:pas * 8 + 8],
                                     zt_ps[:, 0:8])

            def up_tile(st):
                # st in 0..15; q-rows st*128:(st+1)*128; T8 for odd st
                hi = UP_PACKED and (st % 2 == 1)
                lh = slice(64, 128) if hi else slice(0, 64)
                osb = ob.tile([128, D], F16, tag="o", name=f"osb{st}")
                for s2 in range(2):
                    pool = PP if s2 == 0 else PT
                    up = pool.tile([128, SR], F32, tag="p" if s2 == 0 else "t",
                                   name=f"up{st}_{s2}")
                    nc.tensor.matmul(
                        up[:], ctxu[lh, st * JC:(st + 1) * JC],
                        wu_s[lh, s2 * SR:(s2 + 1) * SR],
                        start=True, stop=True)
                    nc.vector.scalar_tensor_tensor(
                        osb[:, s2 * SR:(s2 + 1) * SR], up[:],
                        rzbc[:, st:st + 1], bub_s[:, s2 * SR:(s2 + 1) * SR],
                        mybir.AluOpType.mult, mybir.AluOpType.add)
                # pass-A outs ride sync mid-kernel; tail outs split queues
                # (scalar is free after the last exp, never before)
                eng = nc.sync if (st < 8 or st % 2 == 0) else nc.scalar
                eng.dma_start(out=out_d[st * JC:(st + 1) * JC, :], in_=osb[:])

            # ---- prologue: first ranges chase the DMAs --------------------
            kv_range(0)
            warm(2)
            q_range(0)
            warm(2)
            q_range(1)
            warm(2)
            v_trans(0)
            kv_range(1)
            v_trans(1)
            warm(2)

            # per-(pass, step) PE filler emitted *after* MM1 of that step,
            # placed so no filler waits on DMA later than its position
            fillers = {
                (0, 3): [lambda: kv_range(2), lambda: v_trans(2)],
                (0, 4): [lambda: kv_range(3), lambda: v_trans(3)],
                (0, 5): [lambda: kv_range(4), lambda: v_trans(4)],
                (0, 7): [lambda: q_range(2)],
                (0, 8): [lambda: q_range(3)],
                (1, 0): [bu_bcast],
                (1, 2): [lambda: up_tile(0), lambda: up_tile(1)],
                (1, 3): [lambda: up_tile(2), lambda: up_tile(3)],
                (1, 4): [lambda: up_tile(4), lambda: up_tile(5)],
                (1, 5): [lambda: up_tile(6)],
                (1, 6): [lambda: up_tile(7)],
            }

            for pas in range(2):
                ctx_tiles[pas] = PC.tile([65, QH], F32, tag="c",
                                         name=f"ctx{pas}")
                for si, (ce, co) in enumerate(STEPS):
                    # MM2 lag-1: previous step's chunks
                    if si > 0:
                        pe, po = STEPS[si - 1]
                        mm2(pas, pe)
                        if po is not None:
                            mm2(pas, po)
                    elif pas == 1:
                        # cross-pass: solo chunk of pass A
                        mm2(0, SOLO)
                        ctx_evac(0)
                    mm1_exp(pas, ce, co)
                    for f in fillers.get((pas, si), []):
                        f()
                    if pas == 1 and si == 1:
                        z_recip(0)
                if pas == 1:
                    mm2(1, SOLO)
            ctx_evac(1)
            z_recip(1)
            for st in range(8, 16):
                up_tile(st)

    nc.compile()
    return nc


def get_graph():
    if "graph" not in _CACHE:
        _CACHE["graph"] = build_graph()
    return _CACHE["graph"]


def make_in_maps(x, attention_mask, Wd, bd, Wu, bu):
    # wd2 per k-chunk: [q(64) | q(64) | k | v | v | k]  (384 cols)
    wd2 = np.empty((128, 8 * 384), np.float16)
    for k in range(8):
        blk = Wd[k * 128:(k + 1) * 128, :].astype(np.float16)
        q_, k_, v_ = blk[:, 0:64], blk[:, 64:128], blk[:, 128:192]
        wd2[:, k * 384:(k + 1) * 384] = np.concatenate(
            [q_, q_, k_, v_, v_, k_], axis=1)
    wu2 = np.ascontiguousarray(
        np.concatenate([Wu, Wu], axis=0).astype(mybir.dt.np(mybir.dt.bfloat16)))
    bu_row = np.ascontiguousarray(bu.reshape(1, D).astype(np.float32))
    bdq2 = np.concatenate([bd[0:64], bd[0:64]]).reshape(128, 1).astype(np.float32)
    # col 0: even ranges [k-bias | v-bias]; col 1: odd ranges [v | k]
    bdkv2 = np.stack([
        np.concatenate([bd[64:128], bd[128:192]]),
        np.concatenate([bd[128:192], bd[64:128]]),
    ], axis=1).astype(np.float32)
    per_batch = []
    for b in range(B):
        idx = np.nonzero(attention_mask[b])[0]
        n = len(idx)
        assert n <= K_CAP, f"unmasked key count {n} exceeds K_CAP={K_CAP}"
        idxp = np.concatenate([idx, np.zeros(K_CAP - n, np.int64)])
        xkT = np.ascontiguousarray(
            x[b][idxp].T.astype(np.float16).reshape(8, 128, K_CAP)
            .transpose(1, 0, 2))
        mb = np.full(K_CAP, MASKED_BIAS, np.float32)
        mb[:n] = LOGIT_SHIFT
        per_batch.append((xkT, np.ascontiguousarray(mb.reshape(NJK, 128).T)))
    in_maps = []
    for c in range(N_CORES):
        b, h = c // 2, c % 2
        xkT, mb = per_batch[b]
        xT = np.ascontiguousarray(
            x[b, h * S_LOC:(h + 1) * S_LOC].T.astype(np.float16)
            .reshape(8, 128, S_LOC).transpose(1, 0, 2))
        in_maps.append({
            "xT": xT,
            "xkT": xkT,
            "Wd2": wd2,
            "Wu2": wu2,
            "bu_row": bu_row,
            "bd_q2": bdq2,
            "bd_kv2": bdkv2,
            "maskbias": mb,
        })
    return in_maps


def kernel(x, attention_mask, Wd, bd, Wu, bu):
    from concourse import bass_utils

    x = np.asarray(x, dtype=np.float32)
    attention_mask = np.asarray(attention_mask)
    Wd = np.asarray(Wd, dtype=np.float32)
    bd = np.asarray(bd, dtype=np.float32)
    Wu = np.asarray(Wu, dtype=np.float32)
    bu = np.asarray(bu, dtype=np.float32)

    nc = get_graph()
    in_maps = make_in_maps(x, attention_mask, Wd, bd, Wu, bu)
    res = bass_utils.run_bass_kernel_spmd(nc, in_maps, list(range(N_CORES)))
    out = np.empty((B, S, D), dtype=np.float32)
    for c in range(N_CORES):
        b, h = c // 2, c % 2
        out[b, h * S_LOC:(h + 1) * S_LOC, :] = \
            res.results[c]["out"].astype(np.float32)
    return out


# revision 33
# speedup vs baseline: 1.0978x; 1.0978x over previous
"""Distributed Trainium2 (Bass/Tile) kernel for single-head latent attention.

Reference computation (B=4, S=4096, D=1024, DL=64):
    qkv = x @ Wd + bd; q,k,v = split(qkv)
    logits = (q @ k^T) / sqrt(DL) / TEMP, key-masked
    out = softmax(logits) @ v @ Wu + bu

Sharding: data-parallel over (batch, seq-half) -> 8 shards of 2048 query rows.
Each core recomputes K/V for its batch's keys from x (no collectives).

v2 design (vs the 124us baseline):
  - Fine-grained range-major input DMA on 3 queues; projections chase the
    arriving data and attention pair 0 starts at ~8us instead of ~35us.
  - MM1 row-tiled (contraction DL=64): chunk pairs run concurrently on the
    two 64-row halves of the PE array (tile_position (0,0)/(64,0)).  kT uses
    a parity layout (even kv-ranges' k in partitions 0:64, odd ranges' in
    64:128) produced with per-range-swapped [k|v]/[v|k] projection weights,
    so no cross-partition moves are needed anywhere.  q is duplicated into
    both halves for free by duplicating Wd's q columns (M=64 -> M=128).
  - Postponed normalization: out = (ctxU @ Wu) * (1/Z) + bu.  1/Z is applied
    per-query at PSUM evacuation (scalar_tensor_tensor), ctxU flows bf16
    (unnormalized ctx spans e^+-55 -> overflows fp16 but not bf16).  Z is
    transposed to query-major via 16 tiny PE transposes + one reciprocal.
  - Scalar engine does exp ONLY (34 x [128,1024] ACTIVATEs ~ 34us is the
    serial floor); every copy/evac lives on vector/gpsimd.
  - Up-projection row-tiled too: ctxU duplicated into partitions 64:128 by
    an SBUF->SBUF DMA; odd st tiles run on T8 with Wu's duplicate rows.
  - Dummy matmuls only bridge the initial DMA ramp (HAM clock warm-up).
"""

import sys

if "/opt/trn_rl_repo" not in sys.path:
    sys.path.insert(0, "/opt/trn_rl_repo")

import numpy as np

from concourse import bacc, tile
from concourse import mybir
from concourse.masks import make_identity

F32 = mybir.dt.float32
F32R = mybir.dt.float32r
BF16 = mybir.dt.bfloat16
F16 = mybir.dt.float16

B, S, D, DL = 4, 4096, 1024, 64
N_CORES = 8
S_LOC = S // 2          # 2048 query rows per core
SR = 512
JC = 128                # key chunk
NJK = 17                # compacted key chunks
K_CAP = NJK * JC        # 2176 >= max unmasked keys per batch
QH = 1024               # one attention pass = 1024 query columns
SCALE = 1.25            # 1/sqrt(64)/0.1
LOGIT_SHIFT = -40.0
MASKED_BIAS = -1e30
UP_PACKED = True        # odd st tiles on T8 (needs ctxU dup DMA)

# kv ranges: (col0, width, parity).  Even ranges project with [k|v] weights
# (k -> psum rows 0:64), odd with [v|k] (k -> rows 64:128), so the k-half
# always evacuates same-partition into its kT2 half.
KV_RANGES = [(0, 512, 0), (512, 512, 1), (1024, 512, 0), (1536, 512, 1),
             (2048, 128, 0)]
# chunk -> kT2 half/block: top (rows 0:64) = ranges 0,2,4; bottom = 1,3
TOP_CHUNKS = [0, 1, 2, 3, 8, 9, 10, 11, 16]
BOT_CHUNKS = [4, 5, 6, 7, 12, 13, 14, 15]
# step schedule ordered by DMA arrival (r0 first, r3 last); top/bot pairs
# run concurrently on the two array halves, same-half pairs just serialize
STEPS = [(0, 1), (2, 4), (3, 5), (6, 7), (8, 9), (10, 12), (11, 13),
         (14, 15), (16, None)]
SOLO = 16

_CACHE = {}


def _chunk_block(c):
    """kT2 (half, block) for chunk c."""
    if c in TOP_CHUNKS:
        return 0, TOP_CHUNKS.index(c)
    return 1, BOT_CHUNKS.index(c)


def build_graph():
    nc = bacc.Bacc("TRN2", target_bir_lowering=False, debug=False,
                   num_devices=N_CORES)

    # inputs pre-split host-side into C-contiguous per-transfer blocks so
    # each DMA reads one dense DRAM extent (keeps the HW queues at full BW)
    xk_g, xq_g = {}, {}
    for lo in (0, 4):
        h = "lo" if lo == 0 else "hi"
        xk_g[(lo, 0)] = nc.dram_tensor(
            f"xk_a_{h}", [128, 4, 512], F16, kind="ExternalInput").ap()
        xk_g[(lo, 512)] = nc.dram_tensor(
            f"xk_b_{h}", [128, 4, 512], F16, kind="ExternalInput").ap()
        xk_g[(lo, 1024)] = nc.dram_tensor(
            f"xk_c_{h}", [128, 4, K_CAP - 1024], F16,
            kind="ExternalInput").ap()
        for r in range(4):
            xq_g[(lo, r)] = nc.dram_tensor(
                f"xq_r{r}_{h}", [128, 4, 512], F16, kind="ExternalInput").ap()
    wd_d = nc.dram_tensor("Wd2", [128, 8 * 384], F16, kind="ExternalInput").ap()
    wu_d = nc.dram_tensor("Wu2", [128, D], BF16, kind="ExternalInput").ap()
    bu_d = nc.dram_tensor("bu_row", [1, D], F32, kind="ExternalInput").ap()
    bdq_d = nc.dram_tensor("bd_q2", [128, 1], F32, kind="ExternalInput").ap()
    bdkv_d = nc.dram_tensor("bd_kv2", [128, 2], F32, kind="ExternalInput").ap()
    mb_d = nc.dram_tensor("maskbias", [128, NJK], F32, kind="ExternalInput").ap()
    out_d = nc.dram_tensor("out", [S_LOC, D], F16, kind="ExternalOutput").ap()

    with tile.TileContext(nc) as tc, nc.allow_low_precision(
            reason="bf16/f16 tiles feed full-rate PE matmuls; ~10-bit "
                   "mantissas are far inside the 2e-2 error budget"):
        with (
            tc.tile_pool(name="consts", bufs=1) as consts,
            tc.tile_pool(name="acts", bufs=1) as acts,
            tc.tile_pool(name="ep", bufs=4) as ep,
            tc.tile_pool(name="ob", bufs=3) as ob,
            tc.tile_pool(name="PL", bufs=2, space="PSUM") as PL,
            tc.tile_pool(name="PC", bufs=1, space="PSUM") as PC,
            tc.tile_pool(name="PP", bufs=1, space="PSUM") as PP,
            tc.tile_pool(name="PT", bufs=1, space="PSUM") as PT,
        ):
            # ---- tiny consts built on-device (no DMA dependency) ----------
            seed = consts.tile([128, SR], F16)
            nc.vector.memset(seed[:], 0.25)
            # exp ACT table preload (~2.7us) during the DMA ramp
            actwarm = consts.tile([128, 32], F32)
            nc.scalar.activation(actwarm[:], seed[:, 0:32],
                                 mybir.ActivationFunctionType.Exp)
            identf = consts.tile([128, 64], F32)
            nc.vector.memset(identf[:], 0.0)
            make_identity(nc, identf[0:64, :], nomemset=True)
            make_identity(nc, identf[64:128, :], nomemset=True)
            ident = consts.tile([128, 64], F32R)
            nc.vector.tensor_copy(ident[:], identf[:])
            onesf = consts.tile([128, 4], F32)
            nc.vector.memset(onesf[:], 1.0)
            onesr = consts.tile([128, 4], F32R)
            nc.vector.tensor_copy(onesr[:], onesf[:])
            onesrow = consts.tile([1, 128], F32)
            nc.vector.memset(onesrow[:], 1.0)

            # ---- DMA'd consts (gpsimd queue, small-first) -----------------
            bdq_s = consts.tile([128, 1], F32)
            nc.gpsimd.dma_start(out=bdq_s[:], in_=bdq_d[:])
            bdkv_s = consts.tile([128, 2], F32)
            nc.gpsimd.dma_start(out=bdkv_s[:], in_=bdkv_d[:])
            mb_s = consts.tile([128, NJK], F32)
            nc.gpsimd.dma_start(out=mb_s[:], in_=mb_d[:])
            wd_s = consts.tile([128, 8 * 384], F16)   # DMA'd below (fast q)
            wu_s = consts.tile([128, D], BF16)
            bur_s = consts.tile([1, D], F32)

            # ---- activation tiles -----------------------------------------
            xq_sb = acts.tile([128, 8, S_LOC], F16)
            xk_sb = acts.tile([128, 8, K_CAP], F16)
            qT2 = acts.tile([128, S_LOC], F16)       # q duplicated both halves
            kT2 = acts.tile([128, 9 * JC], F16)      # parity layout
            vTb = acts.tile([128, K_CAP], F32R)      # v staging (half by range)
            v_aug = acts.tile([128, NJK * 65], BF16)  # [v(64) | ones] per chunk
            nc.vector.memset(v_aug[:], 1.0)
            ctxu = acts.tile([128, S_LOC], BF16)     # rows 0:64 ctx, 64:128 dup
            zr = acts.tile([128, S_LOC], F32)        # Z row staging (row 64)
            rzbc = acts.tile([128, 16], F32)         # 1/Z, query-major, col=st
            bub_s = consts.tile([128, D], F32)       # bu broadcast to 128 rows

            # ---- input DMAs: contiguous group blocks on the two fast HW
            # queues (sync lo-slabs, scalar hi-slabs), priority order.
            # gpsimd's queue is software-DGE (slow) -> tiny/late consts only.
            for lo, eng in ((0, nc.sync), (4, nc.scalar)):
                eng.dma_start(out=wd_s[:, lo * 384:(lo + 4) * 384],
                              in_=wd_d[:, lo * 384:(lo + 4) * 384])
                eng.dma_start(out=xk_sb[:, lo:lo + 4, 0:512],
                              in_=xk_g[(lo, 0)][:])
                eng.dma_start(out=xq_sb[:, lo:lo + 4, 0:512],
                              in_=xq_g[(lo, 0)][:])
                eng.dma_start(out=xq_sb[:, lo:lo + 4, 512:1024],
                              in_=xq_g[(lo, 1)][:])
                eng.dma_start(out=xk_sb[:, lo:lo + 4, 512:1024],
                              in_=xk_g[(lo, 512)][:])
                eng.dma_start(out=xk_sb[:, lo:lo + 4, 1024:K_CAP],
                              in_=xk_g[(lo, 1024)][:])
                eng.dma_start(out=xq_sb[:, lo:lo + 4, 1024:1536],
                              in_=xq_g[(lo, 2)][:])
                eng.dma_start(out=xq_sb[:, lo:lo + 4, 1536:2048],
                              in_=xq_g[(lo, 3)][:])
            # wu/bu only needed from pass 1 (~25us): slow queue is fine
            nc.gpsimd.dma_start(out=wu_s[:], in_=wu_d[:])
            nc.gpsimd.dma_start(out=bur_s[:], in_=bu_d[:])

            # ---- helpers --------------------------------------------------
            ndum = [0]

            def warm(n):
                for _ in range(n):
                    dmy = PL.tile([128, QH], F32, tag="L",
                                  name=f"dmy{ndum[0]}")
                    ndum[0] += 1
                    nc.tensor.matmul(dmy[:, 0:SR], seed[:, 0:128],
                                     seed[:], start=True, stop=True)

            def q_range(r):
                ps_q = PP.tile([128, SR], F32, tag="p", name=f"psq{r}")
                for k in range(8):
                    nc.tensor.matmul(
                        ps_q[:], wd_s[:, k * 384:k * 384 + 128],
                        xq_sb[:, k, r * SR:(r + 1) * SR],
                        start=(k == 0), stop=(k == 7))
                nc.vector.tensor_scalar_add(qT2[:, r * SR:(r + 1) * SR],
                                            ps_q[:], bdq_s[:, 0:1])

            def kv_range(ri):
                c0, w, par = KV_RANGES[ri]
                ps_kv = PP.tile([128, SR], F32, tag="p", name=f"pskv{ri}")
                wcol = 128 if par == 0 else 256
                for k in range(8):
                    nc.tensor.matmul(
                        ps_kv[:, 0:w],
                        wd_s[:, k * 384 + wcol:k * 384 + wcol + 128],
                        xk_sb[:, k, c0:c0 + w],
                        start=(k == 0), stop=(k == 7))
                half, blk0 = _chunk_block(c0 // JC)
                kh = slice(0, 64) if half == 0 else slice(64, 128)
                vh = slice(64, 128) if half == 0 else slice(0, 64)
                nc.vector.tensor_scalar_add(
                    kT2[kh, blk0 * JC:blk0 * JC + w], ps_kv[kh, 0:w],
                    bdkv_s[kh, par:par + 1])
                nc.vector.tensor_scalar_add(
                    vTb[vh, c0:c0 + w], ps_kv[vh, 0:w],
                    bdkv_s[vh, par:par + 1])

            def v_trans(ri):
                c0, w, par = KV_RANGES[ri]
                vh = slice(64, 128) if par == 0 else slice(0, 64)
                idh = ident[64:128, :] if par == 0 else ident[0:64, :]
                nch = w // JC
                vt_ps = PT.tile([128, 256], F32R, tag="t", name=f"vt{ri}")
                for j in range(nch):
                    c = c0 // JC + j
                    nc.tensor.transpose(
                        vt_ps[:, j * 64:(j + 1) * 64],
                        vTb[vh, c * JC:(c + 1) * JC], idh)
                for j in range(nch):
                    c = c0 // JC + j
                    nc.vector.tensor_copy(v_aug[:, c * 65:c * 65 + 64],
                                          vt_ps[:, j * 64:(j + 1) * 64])

            def bu_bcast():
                # bu broadcast: [1,1024] -> [128,1024] via two K=1 matmuls
                for s2 in range(2):
                    bu_ps = PT.tile([128, SR], F32, tag="t", name=f"bups{s2}")
                    nc.tensor.matmul(bu_ps[:], onesrow[:, 0:128],
                                     bur_s[:, s2 * SR:(s2 + 1) * SR],
                                     start=True, stop=True)
                    nc.vector.tensor_copy(bub_s[:, s2 * SR:(s2 + 1) * SR],
                                          bu_ps[:])

            warm(3)

            # ================ main software pipeline =======================
            exs = {}            # chunk -> ex tile (per pass, overwritten)
            ctx_tiles = {}
            nmm2 = [0]

            def mm1_exp(pas, ce, co):
                q0 = pas * QH
                lgs = []
                for c in (ce, co):
                    if c is None:
                        continue
                    half, blk = _chunk_block(c)
                    hs = slice(0, 64) if half == 0 else slice(64, 128)
                    lg = PL.tile([128, QH], F32, tag="L",
                                 name=f"lg{pas}_{c}")
                    for s2 in range(2):
                        nc.tensor.matmul(
                            lg[:, s2 * SR:(s2 + 1) * SR],
                            kT2[hs, blk * JC:(blk + 1) * JC],
                            qT2[hs, q0 + s2 * SR:q0 + (s2 + 1) * SR],
                            start=True, stop=True)
                    lgs.append((c, lg))
                for c, lg in lgs:
                    ex = ep.tile([128, QH], BF16, tag="e", name=f"ex{pas}_{c}")
                    nc.scalar.activation(ex[:], lg[:],
                                         mybir.ActivationFunctionType.Exp,
                                         bias=mb_s[:, c:c + 1], scale=SCALE)
                    exs[c] = ex

            def mm2(pas, c):
                ctx_ps = ctx_tiles[pas]
                i = nmm2[0]
                nmm2[0] += 1
                first = (i % NJK == 0)
                last = (i % NJK == NJK - 1)
                for s2 in range(2):
                    nc.tensor.matmul(
                        ctx_ps[:, s2 * SR:(s2 + 1) * SR],
                        v_aug[:, c * 65:(c + 1) * 65],
                        exs[c][:, s2 * SR:(s2 + 1) * SR],
                        start=first, stop=last)

            def ctx_evac(pas):
                q0 = pas * QH
                ctx_ps = ctx_tiles[pas]
                nc.vector.tensor_copy(ctxu[0:64, q0:q0 + QH], ctx_ps[0:64, :])
                nc.vector.tensor_copy(zr[64:65, q0:q0 + QH], ctx_ps[64:65, :])
                if UP_PACKED:
                    nc.sync.dma_start(out=ctxu[64:128, q0:q0 + QH],
                                      in_=ctxu[0:64, q0:q0 + QH])

            def z_recip(pas):
                # transpose Z [1,1024] -> query-major [128,8] via 8 rank-1
                # matmuls (lhsT = Z block, rhs = scalar 1.0), then reciprocal
                q0 = pas * QH
                zt_ps = PT.tile([128, 16], F32, tag="t", name=f"zt{pas}")
                for st in range(8):
                    nc.tensor.matmul(
                        zt_ps[:, st:st + 1],
                        zr[64:65, q0 + st * JC:q0 + (st + 1) * JC],
                        onesf[64:65, 0:1], start=True, stop=True)
                nc.vector.reciprocal(rzbc[:, pas * 8:pas * 8 + 8],
                                     zt_ps[:, 0:8])

            def up_tile(st):
                # st in 0..15; q-rows st*128:(st+1)*128; T8 for odd st
                hi = UP_PACKED and (st % 2 == 1)
                lh = slice(64, 128) if hi else slice(0, 64)
                osb = ob.tile([128, D], F16, tag="o", name=f"osb{st}")
                for s2 in range(2):
                    pool = PP if s2 == 0 else PT
                    up = pool.tile([128, SR], F32, tag="p" if s2 == 0 else "t",
                                   name=f"up{st}_{s2}")
                    nc.tensor.matmul(
                        up[:], ctxu[lh, st * JC:(st + 1) * JC],
                        wu_s[lh, s2 * SR:(s2 + 1) * SR],
                        start=True, stop=True)
                    nc.vector.scalar_tensor_tensor(
                        osb[:, s2 * SR:(s2 + 1) * SR], up[:],
                        rzbc[:, st:st + 1], bub_s[:, s2 * SR:(s2 + 1) * SR],
                        mybir.AluOpType.mult, mybir.AluOpType.add)
                # pass-A outs ride sync mid-kernel; tail outs split queues
                # (scalar is free after the last exp, never before)
                eng = nc.sync if (st < 8 or st % 2 == 0) else nc.scalar
                eng.dma_start(out=out_d[st * JC:(st + 1) * JC, :], in_=osb[:])

            # ---- prologue: first ranges chase the DMAs --------------------
            kv_range(0)
            warm(2)
            q_range(0)
            warm(2)
            q_range(1)
            warm(2)
            v_trans(0)
            kv_range(1)
            v_trans(1)
            warm(2)

            # per-(pass, step) PE filler emitted *after* MM1 of that step,
            # placed so no filler waits on DMA later than its position
            fillers = {
                (0, 3): [lambda: kv_range(2), lambda: v_trans(2)],
                (0, 4): [lambda: kv_range(3), lambda: v_trans(3)],
                (0, 5): [lambda: kv_range(4), lambda: v_trans(4)],
                (0, 7): [lambda: q_range(2)],
                (0, 8): [lambda: q_range(3)],
                (1, 0): [bu_bcast],
                (1, 2): [lambda: up_tile(0), lambda: up_tile(1)],
                (1, 3): [lambda: up_tile(2), lambda: up_tile(3)],
                (1, 4): [lambda: up_tile(4), lambda: up_tile(5)],
                (1, 5): [lambda: up_tile(6)],
                (1, 6): [lambda: up_tile(7)],
            }

            for pas in range(2):
                ctx_tiles[pas] = PC.tile([65, QH], F32, tag="c",
                                         name=f"ctx{pas}")
                for si, (ce, co) in enumerate(STEPS):
                    # MM2 lag-1: previous step's chunks
                    if si > 0:
                        pe, po = STEPS[si - 1]
                        mm2(pas, pe)
                        if po is not None:
                            mm2(pas, po)
                    elif pas == 1:
                        # cross-pass: solo chunk of pass A
                        mm2(0, SOLO)
                        ctx_evac(0)
                    mm1_exp(pas, ce, co)
                    for f in fillers.get((pas, si), []):
                        f()
                    if pas == 1 and si == 1:
                        z_recip(0)
                if pas == 1:
                    mm2(1, SOLO)
            ctx_evac(1)
            z_recip(1)
            for st in range(8, 16):
                up_tile(st)

    nc.compile()
    return nc


def get_graph():
    if "graph" not in _CACHE:
        _CACHE["graph"] = build_graph()
    return _CACHE["graph"]


def make_in_maps(x, attention_mask, Wd, bd, Wu, bu):
    # wd2 per k-chunk: [q(64) | q(64) | k | v | v | k]  (384 cols)
    wd2 = np.empty((128, 8 * 384), np.float16)
    for k in range(8):
        blk = Wd[k * 128:(k + 1) * 128, :].astype(np.float16)
        q_, k_, v_ = blk[:, 0:64], blk[:, 64:128], blk[:, 128:192]
        wd2[:, k * 384:(k + 1) * 384] = np.concatenate(
            [q_, q_, k_, v_, v_, k_], axis=1)
    wu2 = np.ascontiguousarray(
        np.concatenate([Wu, Wu], axis=0).astype(mybir.dt.np(mybir.dt.bfloat16)))
    bu_row = np.ascontiguousarray(bu.reshape(1, D).astype(np.float32))
    bdq2 = np.concatenate([bd[0:64], bd[0:64]]).reshape(128, 1).astype(np.float32)
    # col 0: even ranges [k-bias | v-bias]; col 1: odd ranges [v | k]
    bdkv2 = np.stack([
        np.concatenate([bd[64:128], bd[128:192]]),
        np.concatenate([bd[128:192], bd[64:128]]),
    ], axis=1).astype(np.float32)
    per_batch = []
    for b in range(B):
        idx = np.nonzero(attention_mask[b])[0]
        n = len(idx)
        assert n <= K_CAP, f"unmasked key count {n} exceeds K_CAP={K_CAP}"
        idxp = np.concatenate([idx, np.zeros(K_CAP - n, np.int64)])
        xkT = x[b][idxp].T.astype(np.float16).reshape(
            8, 128, K_CAP).transpose(1, 0, 2)   # [128, slab, col]
        mb = np.full(K_CAP, MASKED_BIAS, np.float32)
        mb[:n] = LOGIT_SHIFT
        per_batch.append((xkT, np.ascontiguousarray(mb.reshape(NJK, 128).T)))
    in_maps = []
    for c in range(N_CORES):
        b, h = c // 2, c % 2
        xkT, mb = per_batch[b]
        xT = x[b, h * S_LOC:(h + 1) * S_LOC].T.astype(np.float16).reshape(
            8, 128, S_LOC).transpose(1, 0, 2)
        m = {
            "Wd2": wd2,
            "Wu2": wu2,
            "bu_row": bu_row,
            "bd_q2": bdq2,
            "bd_kv2": bdkv2,
            "maskbias": mb,
        }
        for lo in (0, 4):
            hh = "lo" if lo == 0 else "hi"
            sl = slice(lo, lo + 4)
            m[f"xk_a_{hh}"] = np.ascontiguousarray(xkT[:, sl, 0:512])
            m[f"xk_b_{hh}"] = np.ascontiguousarray(xkT[:, sl, 512:1024])
            m[f"xk_c_{hh}"] = np.ascontiguousarray(xkT[:, sl, 1024:K_CAP])
            for r in range(4):
                m[f"xq_r{r}_{hh}"] = np.ascontiguousarray(
                    xT[:, sl, r * 512:(r + 1) * 512])
        in_maps.append(m)
    return in_maps


def kernel(x, attention_mask, Wd, bd, Wu, bu):
    from concourse import bass_utils

    x = np.asarray(x, dtype=np.float32)
    attention_mask = np.asarray(attention_mask)
    Wd = np.asarray(Wd, dtype=np.float32)
    bd = np.asarray(bd, dtype=np.float32)
    Wu = np.asarray(Wu, dtype=np.float32)
    bu = np.asarray(bu, dtype=np.float32)

    nc = get_graph()
    in_maps = make_in_maps(x, attention_mask, Wd, bd, Wu, bu)
    res = bass_utils.run_bass_kernel_spmd(nc, in_maps, list(range(N_CORES)))
    out = np.empty((B, S, D), dtype=np.float32)
    for c in range(N_CORES):
        b, h = c // 2, c % 2
        out[b, h * S_LOC:(h + 1) * S_LOC, :] = \
            res.results[c]["out"].astype(np.float32)
    return out


# revision 39
# speedup vs baseline: 1.1384x; 1.0370x over previous
"""Distributed Trainium2 (Bass/Tile) kernel for single-head latent attention.

Reference computation (B=4, S=4096, D=1024, DL=64):
    qkv = x @ Wd + bd; q,k,v = split(qkv)
    logits = (q @ k^T) / sqrt(DL) / TEMP, key-masked
    out = softmax(logits) @ v @ Wu + bu

Sharding: data-parallel over (batch, seq-half) -> 8 shards of 2048 query rows.
Each core recomputes K/V for its batch's keys from x (no collectives).

v2 design (vs the 124us baseline):
  - Fine-grained range-major input DMA on 3 queues; projections chase the
    arriving data and attention pair 0 starts at ~8us instead of ~35us.
  - MM1 row-tiled (contraction DL=64): chunk pairs run concurrently on the
    two 64-row halves of the PE array (tile_position (0,0)/(64,0)).  kT uses
    a parity layout (even kv-ranges' k in partitions 0:64, odd ranges' in
    64:128) produced with per-range-swapped [k|v]/[v|k] projection weights,
    so no cross-partition moves are needed anywhere.  q is duplicated into
    both halves for free by duplicating Wd's q columns (M=64 -> M=128).
  - Postponed normalization: out = (ctxU @ Wu) * (1/Z) + bu.  1/Z is applied
    per-query at PSUM evacuation (scalar_tensor_tensor), ctxU flows bf16
    (unnormalized ctx spans e^+-55 -> overflows fp16 but not bf16).  Z is
    transposed to query-major via 16 tiny PE transposes + one reciprocal.
  - Scalar engine does exp ONLY (34 x [128,1024] ACTIVATEs ~ 34us is the
    serial floor); every copy/evac lives on vector/gpsimd.
  - Up-projection row-tiled too: ctxU duplicated into partitions 64:128 by
    an SBUF->SBUF DMA; odd st tiles run on T8 with Wu's duplicate rows.
  - Dummy matmuls only bridge the initial DMA ramp (HAM clock warm-up).
"""

import sys

if "/opt/trn_rl_repo" not in sys.path:
    sys.path.insert(0, "/opt/trn_rl_repo")

import numpy as np

from concourse import bacc, tile
from concourse import mybir
from concourse.masks import make_identity

F32 = mybir.dt.float32
F32R = mybir.dt.float32r
BF16 = mybir.dt.bfloat16
F16 = mybir.dt.float16

B, S, D, DL = 4, 4096, 1024, 64
N_CORES = 8
S_LOC = S // 2          # 2048 query rows per core
SR = 512
JC = 128                # key chunk
NJK = 17                # compacted key chunks
K_CAP = NJK * JC        # 2176 >= max unmasked keys per batch
QH = 1024               # one attention pass = 1024 query columns
SCALE = 1.25            # 1/sqrt(64)/0.1
LOGIT_SHIFT = -40.0
MASKED_BIAS = -1e30
UP_PACKED = True        # odd st tiles on T8 (needs ctxU dup DMA)

# kv ranges: (col0, width, parity).  Even ranges project with [k|v] weights
# (k -> psum rows 0:64), odd with [v|k] (k -> rows 64:128), so the k-half
# always evacuates same-partition into its kT2 half.
KV_RANGES = [(0, 512, 0), (512, 512, 1), (1024, 512, 0), (1536, 512, 1),
             (2048, 128, 0)]
# chunk -> kT2 half/block: top (rows 0:64) = ranges 0,2,4; bottom = 1,3
TOP_CHUNKS = [0, 1, 2, 3, 8, 9, 10, 11, 16]
BOT_CHUNKS = [4, 5, 6, 7, 12, 13, 14, 15]
# step schedule ordered by DMA arrival (r0 first, r3 last); top/bot pairs
# run concurrently on the two array halves, same-half pairs just serialize
STEPS = [(0, 1), (2, 4), (3, 5), (6, 7), (8, 9), (10, 12), (11, 13),
         (14, 15), (16, None)]
SOLO = 16

_CACHE = {}


def _chunk_block(c):
    """kT2 (half, block) for chunk c."""
    if c in TOP_CHUNKS:
        return 0, TOP_CHUNKS.index(c)
    return 1, BOT_CHUNKS.index(c)


def build_graph():
    nc = bacc.Bacc("TRN2", target_bir_lowering=False, debug=False,
                   num_devices=N_CORES)

    # inputs pre-split host-side into C-contiguous per-transfer blocks; the
    # SBUF side is laid out identically (group-blocked) so BOTH ends of each
    # DMA are 4KB+/partition contiguous -> big packets -> full queue BW
    xk_g, xq_g = {}, {}
    for lo in (0, 4):
        h = "lo" if lo == 0 else "hi"
        xk_g[(lo, 0)] = nc.dram_tensor(
            f"xk_a_{h}", [128, 4 * 512], F16, kind="ExternalInput").ap()
        xk_g[(lo, 512)] = nc.dram_tensor(
            f"xk_b_{h}", [128, 4 * 512], F16, kind="ExternalInput").ap()
        xk_g[(lo, 1024)] = nc.dram_tensor(
            f"xk_c_{h}", [128, 4 * (K_CAP - 1024)], F16,
            kind="ExternalInput").ap()
        for r in range(4):
            xq_g[(lo, r)] = nc.dram_tensor(
                f"xq_r{r}_{h}", [128, 4 * 512], F16,
                kind="ExternalInput").ap()

    def xk_col(k, c):
        # flat xk_sb column for slab k, key-col c (group-blocked layout)
        half, kl = k // 4, k % 4
        if c < 512:
            return half * 2048 + kl * 512 + c
        if c < 1024:
            return 4096 + half * 2048 + kl * 512 + (c - 512)
        return 8192 + half * 4608 + kl * 1152 + (c - 1024)

    def xq_col(k, c):
        half, kl = k // 4, k % 4
        return (c // 512) * 4096 + half * 2048 + kl * 512 + (c % 512)
    wd_d = nc.dram_tensor("Wd2", [128, 8 * 384], F16, kind="ExternalInput").ap()
    wu_d = nc.dram_tensor("Wu2", [128, D], BF16, kind="ExternalInput").ap()
    bu_d = nc.dram_tensor("bu_row", [1, D], F32, kind="ExternalInput").ap()
    bdq_d = nc.dram_tensor("bd_q2", [128, 1], F32, kind="ExternalInput").ap()
    bdkv_d = nc.dram_tensor("bd_kv2", [128, 2], F32, kind="ExternalInput").ap()
    mb_d = nc.dram_tensor("maskbias", [128, NJK], F32, kind="ExternalInput").ap()
    out_d = nc.dram_tensor("out", [S_LOC, D], F16, kind="ExternalOutput").ap()

    with tile.TileContext(nc) as tc, nc.allow_low_precision(
            reason="bf16/f16 tiles feed full-rate PE matmuls; ~10-bit "
                   "mantissas are far inside the 2e-2 error budget"):
        with (
            tc.tile_pool(name="consts", bufs=1) as consts,
            tc.tile_pool(name="acts", bufs=1) as acts,
            tc.tile_pool(name="ep", bufs=4) as ep,
            tc.tile_pool(name="ob", bufs=3) as ob,
            tc.tile_pool(name="PL", bufs=2, space="PSUM") as PL,
            tc.tile_pool(name="PC", bufs=1, space="PSUM") as PC,
            tc.tile_pool(name="PP", bufs=1, space="PSUM") as PP,
            tc.tile_pool(name="PT", bufs=1, space="PSUM") as PT,
        ):
            # ---- tiny consts built on-device (no DMA dependency) ----------
            seed = consts.tile([128, SR], F16)
            nc.vector.memset(seed[:], 0.25)
            # exp ACT table preload (~2.7us) during the DMA ramp
            actwarm = consts.tile([128, 32], F32)
            nc.scalar.activation(actwarm[:], seed[:, 0:32],
                                 mybir.ActivationFunctionType.Exp)
            identf = consts.tile([128, 64], F32)
            nc.vector.memset(identf[:], 0.0)
            make_identity(nc, identf[0:64, :], nomemset=True)
            make_identity(nc, identf[64:128, :], nomemset=True)
            ident = consts.tile([128, 64], F32R)
            nc.vector.tensor_copy(ident[:], identf[:])
            onesf = consts.tile([128, 4], F32)
            nc.vector.memset(onesf[:], 1.0)
            onesr = consts.tile([128, 4], F32R)
            nc.vector.tensor_copy(onesr[:], onesf[:])
            onesrow = consts.tile([1, 128], F32)
            nc.vector.memset(onesrow[:], 1.0)

            # ---- DMA'd consts (gpsimd queue, small-first) -----------------
            bdq_s = consts.tile([128, 1], F32)
            nc.gpsimd.dma_start(out=bdq_s[:], in_=bdq_d[:])
            bdkv_s = consts.tile([128, 2], F32)
            nc.gpsimd.dma_start(out=bdkv_s[:], in_=bdkv_d[:])
            mb_s = consts.tile([128, NJK], F32)
            nc.gpsimd.dma_start(out=mb_s[:], in_=mb_d[:])
            wd_s = consts.tile([128, 8 * 384], F16)   # DMA'd below (fast q)
            wu_s = consts.tile([128, D], BF16)
            bur_s = consts.tile([1, D], F32)

            # ---- activation tiles (x slabs group-blocked, see xk_col) -----
            xq_sb = acts.tile([128, 8 * S_LOC], F16)
            xk_sb = acts.tile([128, 8 * K_CAP], F16)
            qT2 = acts.tile([128, S_LOC], F16)       # q duplicated both halves
            kT2 = acts.tile([128, 9 * JC], F16)      # parity layout
            vTb = acts.tile([128, K_CAP], F32R)      # v staging (half by range)
            v_aug = acts.tile([128, NJK * 65], BF16)  # [v(64) | ones] per chunk
            nc.vector.memset(v_aug[:], 1.0)
            ctxu = acts.tile([128, S_LOC], BF16)     # rows 0:64 ctx, 64:128 dup
            zr = acts.tile([128, S_LOC], F32)        # Z row staging (row 64)
            rzbc = acts.tile([128, 16], F32)         # 1/Z, query-major, col=st
            bub_s = consts.tile([128, D], F32)       # bu broadcast to 128 rows

            # ---- input DMAs: contiguous group blocks on the two fast HW
            # queues (sync lo-slabs, scalar hi-slabs), priority order.
            # gpsimd's queue is software-DGE (slow) -> tiny/late consts only.
            for lo, eng in ((0, nc.sync), (4, nc.scalar)):
                eng.dma_start(out=wd_s[:, lo * 384:(lo + 4) * 384],
                              in_=wd_d[:, lo * 384:(lo + 4) * 384])
                c = xk_col(lo, 0)
                eng.dma_start(out=xk_sb[:, c:c + 2048], in_=xk_g[(lo, 0)][:])
                c = xq_col(lo, 0)
                eng.dma_start(out=xq_sb[:, c:c + 2048], in_=xq_g[(lo, 0)][:])
                c = xq_col(lo, 512)
                eng.dma_start(out=xq_sb[:, c:c + 2048], in_=xq_g[(lo, 1)][:])
                c = xk_col(lo, 512)
                eng.dma_start(out=xk_sb[:, c:c + 2048], in_=xk_g[(lo, 512)][:])
                c = xk_col(lo, 1024)
                eng.dma_start(out=xk_sb[:, c:c + 4608],
                              in_=xk_g[(lo, 1024)][:])
                c = xq_col(lo, 1024)
                eng.dma_start(out=xq_sb[:, c:c + 2048], in_=xq_g[(lo, 2)][:])
                c = xq_col(lo, 1536)
                eng.dma_start(out=xq_sb[:, c:c + 2048], in_=xq_g[(lo, 3)][:])
            # wu/bu only needed from pass 1 (~25us): slow queue is fine
            nc.gpsimd.dma_start(out=wu_s[:], in_=wu_d[:])
            nc.gpsimd.dma_start(out=bur_s[:], in_=bu_d[:])

            # ---- helpers --------------------------------------------------
            ndum = [0]

            def warm(n):
                for _ in range(n):
                    dmy = PL.tile([128, QH], F32, tag="L",
                                  name=f"dmy{ndum[0]}")
                    ndum[0] += 1
                    nc.tensor.matmul(dmy[:, 0:SR], seed[:, 0:128],
                                     seed[:], start=True, stop=True)

            def q_range(r):
                ps_q = PP.tile([128, SR], F32, tag="p", name=f"psq{r}")
                for k in range(8):
                    c = xq_col(k, r * SR)
                    nc.tensor.matmul(
                        ps_q[:], wd_s[:, k * 384:k * 384 + 128],
                        xq_sb[:, c:c + SR],
                        start=(k == 0), stop=(k == 7))
                nc.vector.tensor_scalar_add(qT2[:, r * SR:(r + 1) * SR],
                                            ps_q[:], bdq_s[:, 0:1])

            def kv_range(ri):
                c0, w, par = KV_RANGES[ri]
                ps_kv = PP.tile([128, SR], F32, tag="p", name=f"pskv{ri}")
                wcol = 128 if par == 0 else 256
                for k in range(8):
                    c = xk_col(k, c0)
                    nc.tensor.matmul(
                        ps_kv[:, 0:w],
                        wd_s[:, k * 384 + wcol:k * 384 + wcol + 128],
                        xk_sb[:, c:c + w],
                        start=(k == 0), stop=(k == 7))
                half, blk0 = _chunk_block(c0 // JC)
                kh = slice(0, 64) if half == 0 else slice(64, 128)
                vh = slice(64, 128) if half == 0 else slice(0, 64)
                nc.vector.tensor_scalar_add(
                    kT2[kh, blk0 * JC:blk0 * JC + w], ps_kv[kh, 0:w],
                    bdkv_s[kh, par:par + 1])
                nc.vector.tensor_scalar_add(
                    vTb[vh, c0:c0 + w], ps_kv[vh, 0:w],
                    bdkv_s[vh, par:par + 1])

            def v_trans(ri):
                c0, w, par = KV_RANGES[ri]
                vh = slice(64, 128) if par == 0 else slice(0, 64)
                idh = ident[64:128, :] if par == 0 else ident[0:64, :]
                nch = w // JC
                vt_ps = PT.tile([128, 256], F32R, tag="t", name=f"vt{ri}")
                for j in range(nch):
                    c = c0 // JC + j
                    nc.tensor.transpose(
                        vt_ps[:, j * 64:(j + 1) * 64],
                        vTb[vh, c * JC:(c + 1) * JC], idh)
                for j in range(nch):
                    c = c0 // JC + j
                    nc.vector.tensor_copy(v_aug[:, c * 65:c * 65 + 64],
                                          vt_ps[:, j * 64:(j + 1) * 64])

            def bu_bcast():
                # bu broadcast: [1,1024] -> [128,1024] via two K=1 matmuls
                for s2 in range(2):
                    bu_ps = PT.tile([128, SR], F32, tag="t", name=f"bups{s2}")
                    nc.tensor.matmul(bu_ps[:], onesrow[:, 0:128],
                                     bur_s[:, s2 * SR:(s2 + 1) * SR],
                                     start=True, stop=True)
                    nc.vector.tensor_copy(bub_s[:, s2 * SR:(s2 + 1) * SR],
                                          bu_ps[:])

            warm(3)

            # ================ main software pipeline =======================
            exs = {}            # chunk -> ex tile (per pass, overwritten)
            ctx_tiles = {}
            nmm2 = [0]

            def mm1_exp(pas, ce, co):
                q0 = pas * QH
                lgs = []
                for c in (ce, co):
                    if c is None:
                        continue
                    half, blk = _chunk_block(c)
                    hs = slice(0, 64) if half == 0 else slice(64, 128)
                    lg = PL.tile([128, QH], F32, tag="L",
                                 name=f"lg{pas}_{c}")
                    for s2 in range(2):
                        nc.tensor.matmul(
                            lg[:, s2 * SR:(s2 + 1) * SR],
                            kT2[hs, blk * JC:(blk + 1) * JC],
                            qT2[hs, q0 + s2 * SR:q0 + (s2 + 1) * SR],
                            start=True, stop=True)
                    lgs.append((c, lg))
                for c, lg in lgs:
                    ex = ep.tile([128, QH], BF16, tag="e", name=f"ex{pas}_{c}")
                    nc.scalar.activation(ex[:], lg[:],
                                         mybir.ActivationFunctionType.Exp,
                                         bias=mb_s[:, c:c + 1], scale=SCALE)
                    exs[c] = ex

            def mm2(pas, c):
                ctx_ps = ctx_tiles[pas]
                i = nmm2[0]
                nmm2[0] += 1
                first = (i % NJK == 0)
                last = (i % NJK == NJK - 1)
                for s2 in range(2):
                    nc.tensor.matmul(
                        ctx_ps[:, s2 * SR:(s2 + 1) * SR],
                        v_aug[:, c * 65:(c + 1) * 65],
                        exs[c][:, s2 * SR:(s2 + 1) * SR],
                        start=first, stop=last)

            def ctx_evac(pas):
                q0 = pas * QH
                ctx_ps = ctx_tiles[pas]
                nc.vector.tensor_copy(ctxu[0:64, q0:q0 + QH], ctx_ps[0:64, :])
                nc.vector.tensor_copy(zr[64:65, q0:q0 + QH], ctx_ps[64:65, :])
                if UP_PACKED:
                    nc.sync.dma_start(out=ctxu[64:128, q0:q0 + QH],
                                      in_=ctxu[0:64, q0:q0 + QH])

            def z_recip(pas):
                # transpose Z [1,1024] -> query-major [128,8] via 8 rank-1
                # matmuls (lhsT = Z block, rhs = scalar 1.0), then reciprocal
                q0 = pas * QH
                zt_ps = PT.tile([128, 16], F32, tag="t", name=f"zt{pas}")
                for st in range(8):
                    nc.tensor.matmul(
                        zt_ps[:, st:st + 1],
                        zr[64:65, q0 + st * JC:q0 + (st + 1) * JC],
                        onesf[64:65, 0:1], start=True, stop=True)
                nc.vector.reciprocal(rzbc[:, pas * 8:pas * 8 + 8],
                                     zt_ps[:, 0:8])

            def up_tile(st):
                # st in 0..15; q-rows st*128:(st+1)*128; T8 for odd st
                hi = UP_PACKED and (st % 2 == 1)
                lh = slice(64, 128) if hi else slice(0, 64)
                osb = ob.tile([128, D], F16, tag="o", name=f"osb{st}")
                for s2 in range(2):
                    pool = PP if s2 == 0 else PT
                    up = pool.tile([128, SR], F32, tag="p" if s2 == 0 else "t",
                                   name=f"up{st}_{s2}")
                    nc.tensor.matmul(
                        up[:], ctxu[lh, st * JC:(st + 1) * JC],
                        wu_s[lh, s2 * SR:(s2 + 1) * SR],
                        start=True, stop=True)
                    nc.vector.scalar_tensor_tensor(
                        osb[:, s2 * SR:(s2 + 1) * SR], up[:],
                        rzbc[:, st:st + 1], bub_s[:, s2 * SR:(s2 + 1) * SR],
                        mybir.AluOpType.mult, mybir.AluOpType.add)
                # pass-A outs ride sync mid-kernel; tail outs split queues
                # (scalar is free after the last exp, never before)
                eng = nc.sync if (st < 8 or st % 2 == 0) else nc.scalar
                eng.dma_start(out=out_d[st * JC:(st + 1) * JC, :], in_=osb[:])

            # ---- prologue: first ranges chase the DMAs --------------------
            kv_range(0)
            warm(2)
            q_range(0)
            warm(2)
            q_range(1)
            warm(2)
            v_trans(0)
            kv_range(1)
            v_trans(1)
            warm(2)

            # per-(pass, step) PE filler emitted *after* MM1 of that step,
            # placed so no filler waits on DMA later than its position
            fillers = {
                (0, 3): [lambda: kv_range(2), lambda: v_trans(2)],
                (0, 4): [lambda: kv_range(3), lambda: v_trans(3)],
                (0, 5): [lambda: kv_range(4), lambda: v_trans(4)],
                (0, 7): [lambda: q_range(2)],
                (0, 8): [lambda: q_range(3)],
                (1, 0): [bu_bcast],
                (1, 2): [lambda: up_tile(0), lambda: up_tile(1)],
                (1, 3): [lambda: up_tile(2), lambda: up_tile(3)],
                (1, 4): [lambda: up_tile(4), lambda: up_tile(5)],
                (1, 5): [lambda: up_tile(6)],
                (1, 6): [lambda: up_tile(7)],
            }

            for pas in range(2):
                ctx_tiles[pas] = PC.tile([65, QH], F32, tag="c",
                                         name=f"ctx{pas}")
                for si, (ce, co) in enumerate(STEPS):
                    # MM2 lag-1: previous step's chunks
                    if si > 0:
                        pe, po = STEPS[si - 1]
                        mm2(pas, pe)
                        if po is not None:
                            mm2(pas, po)
                    elif pas == 1:
                        # cross-pass: solo chunk of pass A
                        mm2(0, SOLO)
                        ctx_evac(0)
                    mm1_exp(pas, ce, co)
                    for f in fillers.get((pas, si), []):
                        f()
                    if pas == 1 and si == 1:
                        z_recip(0)
                if pas == 1:
                    mm2(1, SOLO)
            ctx_evac(1)
            z_recip(1)
            for st in range(8, 16):
                up_tile(st)

    nc.compile()
    return nc


def get_graph():
    if "graph" not in _CACHE:
        _CACHE["graph"] = build_graph()
    return _CACHE["graph"]


def make_in_maps(x, attention_mask, Wd, bd, Wu, bu):
    # wd2 per k-chunk: [q(64) | q(64) | k | v | v | k]  (384 cols)
    wd2 = np.empty((128, 8 * 384), np.float16)
    for k in range(8):
        blk = Wd[k * 128:(k + 1) * 128, :].astype(np.float16)
        q_, k_, v_ = blk[:, 0:64], blk[:, 64:128], blk[:, 128:192]
        wd2[:, k * 384:(k + 1) * 384] = np.concatenate(
            [q_, q_, k_, v_, v_, k_], axis=1)
    wu2 = np.ascontiguousarray(
        np.concatenate([Wu, Wu], axis=0).astype(mybir.dt.np(mybir.dt.bfloat16)))
    bu_row = np.ascontiguousarray(bu.reshape(1, D).astype(np.float32))
    bdq2 = np.concatenate([bd[0:64], bd[0:64]]).reshape(128, 1).astype(np.float32)
    # col 0: even ranges [k-bias | v-bias]; col 1: odd ranges [v | k]
    bdkv2 = np.stack([
        np.concatenate([bd[64:128], bd[128:192]]),
        np.concatenate([bd[128:192], bd[64:128]]),
    ], axis=1).astype(np.float32)
    per_batch = []
    for b in range(B):
        idx = np.nonzero(attention_mask[b])[0]
        n = len(idx)
        assert n <= K_CAP, f"unmasked key count {n} exceeds K_CAP={K_CAP}"
        idxp = np.concatenate([idx, np.zeros(K_CAP - n, np.int64)])
        xkT = x[b][idxp].T.astype(np.float16).reshape(
            8, 128, K_CAP).transpose(1, 0, 2)   # [128, slab, col]
        mb = np.full(K_CAP, MASKED_BIAS, np.float32)
        mb[:n] = LOGIT_SHIFT
        per_batch.append((xkT, np.ascontiguousarray(mb.reshape(NJK, 128).T)))
    in_maps = []
    for c in range(N_CORES):
        b, h = c // 2, c % 2
        xkT, mb = per_batch[b]
        xT = x[b, h * S_LOC:(h + 1) * S_LOC].T.astype(np.float16).reshape(
            8, 128, S_LOC).transpose(1, 0, 2)
        m = {
            "Wd2": wd2,
            "Wu2": wu2,
            "bu_row": bu_row,
            "bd_q2": bdq2,
            "bd_kv2": bdkv2,
            "maskbias": mb,
        }
        for lo in (0, 4):
            hh = "lo" if lo == 0 else "hi"
            sl = slice(lo, lo + 4)
            m[f"xk_a_{hh}"] = np.ascontiguousarray(
                xkT[:, sl, 0:512]).reshape(128, -1)
            m[f"xk_b_{hh}"] = np.ascontiguousarray(
                xkT[:, sl, 512:1024]).reshape(128, -1)
            m[f"xk_c_{hh}"] = np.ascontiguousarray(
                xkT[:, sl, 1024:K_CAP]).reshape(128, -1)
            for r in range(4):
                m[f"xq_r{r}_{hh}"] = np.ascontiguousarray(
                    xT[:, sl, r * 512:(r + 1) * 512]).reshape(128, -1)
        in_maps.append(m)
    return in_maps


def kernel(x, attention_mask, Wd, bd, Wu, bu):
    from concourse import bass_utils

    x = np.asarray(x, dtype=np.float32)
    attention_mask = np.asarray(attention_mask)
    Wd = np.asarray(Wd, dtype=np.float32)
    bd = np.asarray(bd, dtype=np.float32)
    Wu = np.asarray(Wu, dtype=np.float32)
    bu = np.asarray(bu, dtype=np.float32)

    nc = get_graph()
    in_maps = make_in_maps(x, attention_mask, Wd, bd, Wu, bu)
    res = bass_utils.run_bass_kernel_spmd(nc, in_maps, list(range(N_CORES)))
    out = np.empty((B, S, D), dtype=np.float32)
    for c in range(N_CORES):
        b, h = c // 2, c % 2
        out[b, h * S_LOC:(h + 1) * S_LOC, :] = \
            res.results[c]["out"].astype(np.float32)
    return out


# revision 46
# speedup vs baseline: 1.1925x; 1.0475x over previous
"""Distributed Trainium2 (Bass/Tile) kernel for single-head latent attention.

Reference computation (B=4, S=4096, D=1024, DL=64):
    qkv = x @ Wd + bd; q,k,v = split(qkv)
    logits = (q @ k^T) / sqrt(DL) / TEMP, key-masked
    out = softmax(logits) @ v @ Wu + bu

Sharding: data-parallel over (batch, seq-half) -> 8 shards of 2048 query
rows; each core recomputes K/V for its batch's compacted keys (no
collectives).

Design notes (vs the 124us baseline):
  - Host-side mask compaction (only ~2048 unmasked keys kept, cap 2176).
  - Inputs are pre-split host-side into C-contiguous per-transfer blocks and
    the SBUF side is laid out identically, so both ends of every DMA are
    4KB+/partition contiguous -> big packets -> full HW-queue bandwidth.
    Only sync + scalar have hardware DGE queues; gpsimd's is ~10x slower
    (software DGE) and carries only tiny consts.
  - Projections chase the arriving range blocks; attention starts while
    later ranges are still in flight.  Dummy matmuls bridge every DMA stall
    so the PE HAM clock-gate stays at 2.4 GHz.
  - MM1 row-tiled (contraction DL=64): chunk pairs run concurrently on the
    two 64-row halves of the PE array.  kT has a parity layout (even
    kv-ranges' k in partitions 0:64, odd in 64:128) produced with
    per-range-swapped [k|v]/[v|k] projection weights so no cross-partition
    moves are needed; q is duplicated into both halves for free by
    duplicating Wd's q columns (M=64 -> M=128 projection).
  - Postponed normalization: out = (ctxU @ [Wu; bu*Z]) * (1/Z) per query.
    MM2's stationary is [v | ones], so ctx rows 0:64 are unnormalized ctx
    and row 64 is Z; the up-projection contracts over 65 rows (row 64 = bu)
    and the PSUM evacuation is a bias-free per-partition scale by 1/Z that
    either the vector OR scalar engine can apply (tail splits across both).
    ctxU flows bf16 (values span e^+-55: overflows fp16, not bf16).
  - Z is transposed to query-major via 8 tiny f32r PE transposes per pass +
    one reciprocal; scalar engine otherwise does exp ONLY (34 [128,1024]
    ACTIVATEs ~ 38us is the pacing floor).
"""

import sys

if "/opt/trn_rl_repo" not in sys.path:
    sys.path.insert(0, "/opt/trn_rl_repo")

import numpy as np

from concourse import bacc, tile
from concourse import mybir
from concourse.masks import make_identity

F32 = mybir.dt.float32
F32R = mybir.dt.float32r
BF16 = mybir.dt.bfloat16
F16 = mybir.dt.float16

B, S, D, DL = 4, 4096, 1024, 64
N_CORES = 8
S_LOC = S // 2          # 2048 query rows per core
SR = 512
JC = 128                # key chunk
NJK = 17                # compacted key chunks
K_CAP = NJK * JC        # 2176 >= max unmasked keys per batch
QH = 1024               # one attention pass = 1024 query columns
SCALE = 1.25            # 1/sqrt(64)/0.1
LOGIT_SHIFT = -40.0
MASKED_BIAS = -1e30

# kv ranges: (col0, width, parity).  Even ranges project with [k|v] weights
# (k -> psum rows 0:64), odd with [v|k] (k -> rows 64:128), so the k-half
# always evacuates same-partition into its kT2 half.
KV_RANGES = [(0, 512, 0), (512, 512, 1), (1024, 512, 0), (1536, 512, 1),
             (2048, 128, 0)]
# chunk -> kT2 half/block: top (rows 0:64) = ranges 0,2,4; bottom = 1,3
TOP_CHUNKS = [0, 1, 2, 3, 8, 9, 10, 11, 16]
BOT_CHUNKS = [4, 5, 6, 7, 12, 13, 14, 15]
# pass A: step order follows DMA arrival (range 0 chunks first); pass B has
# all data resident -> strict top/bot pairs for full MM1 concurrency
STEPS_A = [(0, 1), (2, 4), (3, 5), (6, 7), (8, 9), (10, 12), (11, 13),
           (14, 15), (16, None)]
STEPS_B = [(0, 4), (1, 5), (2, 6), (3, 7), (8, 12), (9, 13), (10, 14),
           (11, 15), (16, None)]
SOLO = 16

_CACHE = {}


def _chunk_block(c):
    """kT2 (half, block) for chunk c."""
    if c in TOP_CHUNKS:
        return 0, TOP_CHUNKS.index(c)
    return 1, BOT_CHUNKS.index(c)


def build_graph():
    nc = bacc.Bacc("TRN2", target_bir_lowering=False, debug=False,
                   num_devices=N_CORES)

    # Wd packed as: cols 0:2048 = per-chunk [q|q|k|v] (256 each, the only
    # part needed before attention starts), cols 2048:3072 = per-chunk [v|k]
    wd_d = nc.dram_tensor("Wd2", [128, 3072], F16, kind="ExternalInput").ap()
    wu_d = nc.dram_tensor("Wu2", [65, D], BF16, kind="ExternalInput").ap()
    bdq_d = nc.dram_tensor("bd_q2", [128, 1], F32, kind="ExternalInput").ap()
    bdkv_d = nc.dram_tensor("bd_kv2", [128, 2], F32, kind="ExternalInput").ap()
    mb_d = nc.dram_tensor("maskbias", [128, NJK], F32, kind="ExternalInput").ap()
    out_d = nc.dram_tensor("out", [S_LOC, D], F16, kind="ExternalOutput").ap()

    xk_g, xq_g = {}, {}
    for lo in (0, 4):
        h = "lo" if lo == 0 else "hi"
        xk_g[(lo, 0)] = nc.dram_tensor(
            f"xk_a_{h}", [128, 4 * 512], F16, kind="ExternalInput").ap()
        xk_g[(lo, 512)] = nc.dram_tensor(
            f"xk_b_{h}", [128, 4 * 512], F16, kind="ExternalInput").ap()
        xk_g[(lo, 1024)] = nc.dram_tensor(
            f"xk_c_{h}", [128, 4 * (K_CAP - 1024)], F16,
            kind="ExternalInput").ap()
        for r in range(4):
            xq_g[(lo, r)] = nc.dram_tensor(
                f"xq_r{r}_{h}", [128, 4 * 512], F16,
                kind="ExternalInput").ap()

    def xk_col(k, c):
        # flat xk_sb column for slab k, key-col c (group-blocked layout)
        half, kl = k // 4, k % 4
        if c < 512:
            return half * 2048 + kl * 512 + c
        if c < 1024:
            return 4096 + half * 2048 + kl * 512 + (c - 512)
        return 8192 + half * 4608 + kl * 1152 + (c - 1024)

    def xq_col(k, c):
        half, kl = k // 4, k % 4
        return (c // 512) * 4096 + half * 2048 + kl * 512 + (c % 512)

    with tile.TileContext(nc) as tc, nc.allow_low_precision(
            reason="bf16/f16 tiles feed full-rate PE matmuls; ~10-bit "
                   "mantissas are far inside the 2e-2 error budget"):
        with (
            tc.tile_pool(name="consts", bufs=1) as consts,
            tc.tile_pool(name="acts", bufs=1) as acts,
            tc.tile_pool(name="ep", bufs=4) as ep,
            tc.tile_pool(name="ob", bufs=3) as ob,
            tc.tile_pool(name="PL", bufs=2, space="PSUM") as PL,
            tc.tile_pool(name="PC", bufs=1, space="PSUM") as PC,
            tc.tile_pool(name="PP", bufs=1, space="PSUM") as PP,
            tc.tile_pool(name="PT", bufs=1, space="PSUM") as PT,
        ):
            # ---- warm-up seeds, all on the scalar engine (its preamble is
            # the shortest) so PE dummies + ACT table load start ~1us ------
            seed = consts.tile([128, SR], F16)
            nc.scalar.memzero(seed[:])
            actwarm = consts.tile([128, 32], F32)
            nc.scalar.activation(actwarm[:], seed[:, 0:32],
                                 mybir.ActivationFunctionType.Exp)

            # ---- small consts ---------------------------------------------
            identf = consts.tile([128, 64], F32)
            nc.vector.memset(identf[:], 0.0)
            make_identity(nc, identf[0:64, :], nomemset=True)
            make_identity(nc, identf[64:128, :], nomemset=True)
            ident = consts.tile([128, 64], F32R)
            nc.vector.tensor_copy(ident[:], identf[:])
            idzf = consts.tile([128, 4], F32)
            nc.vector.memset(idzf[:], 0.0)
            make_identity(nc, idzf[64:66, 0:2], nomemset=True)
            idz = consts.tile([128, 4], F32R)
            nc.vector.tensor_copy(idz[:], idzf[:])

            # ---- DMA'd consts (gpsimd slow queue: tiny / late-needed) -----
            bdq_s = consts.tile([128, 1], F32)
            nc.gpsimd.dma_start(out=bdq_s[:], in_=bdq_d[:])
            bdkv_s = consts.tile([128, 2], F32)
            nc.gpsimd.dma_start(out=bdkv_s[:], in_=bdkv_d[:])
            mb_s = consts.tile([128, NJK], F32)
            nc.gpsimd.dma_start(out=mb_s[:], in_=mb_d[:])
            wu_s = consts.tile([65, D], BF16)
            nc.gpsimd.dma_start(out=wu_s[:], in_=wu_d[:])
            wd_s = consts.tile([128, 3072], F16)

            # ---- activation tiles (x slabs group-blocked, see xk_col) -----
            xq_sb = acts.tile([128, 8 * S_LOC], F16)
            xk_sb = acts.tile([128, 8 * K_CAP], F16)
            qT2 = acts.tile([128, S_LOC], F16)       # q in both halves
            kT2 = acts.tile([128, 9 * JC], F16)      # parity layout
            vTb = acts.tile([128, K_CAP], F32R)      # v staging (half by rng)
            v_aug = acts.tile([128, NJK * 65], BF16)  # [v(64)|ones] per chunk
            nc.vector.memset(v_aug[:], 1.0)
            ctxu = acts.tile([65, S_LOC], BF16)      # rows 0:64 ctx, 64 = Z
            zr = acts.tile([128, S_LOC], F32R)       # Z row staging (row 64)
            rzbc = acts.tile([128, 32], F32)         # 1/Z query-major, 2*st

            # ---- input DMAs: contiguous blocks, two fast queues -----------
            for lo, eng in ((0, nc.sync), (4, nc.scalar)):
                eng.dma_start(out=wd_s[:, lo * 256:(lo + 4) * 256],
                              in_=wd_d[:, lo * 256:(lo + 4) * 256])
                c = xk_col(lo, 0)
                eng.dma_start(out=xk_sb[:, c:c + 2048], in_=xk_g[(lo, 0)][:])
                c = xq_col(lo, 0)
                eng.dma_start(out=xq_sb[:, c:c + 2048], in_=xq_g[(lo, 0)][:])
                c = xq_col(lo, 512)
                eng.dma_start(out=xq_sb[:, c:c + 2048], in_=xq_g[(lo, 1)][:])
                # odd-range [v|k] weight block
                eng.dma_start(out=wd_s[:, 2048 + lo * 128:2048 + (lo + 4) * 128],
                              in_=wd_d[:, 2048 + lo * 128:2048 + (lo + 4) * 128])
                c = xk_col(lo, 512)
                eng.dma_start(out=xk_sb[:, c:c + 2048], in_=xk_g[(lo, 512)][:])
                c = xk_col(lo, 1024)
                eng.dma_start(out=xk_sb[:, c:c + 4608],
                              in_=xk_g[(lo, 1024)][:])
                c = xq_col(lo, 1024)
                eng.dma_start(out=xq_sb[:, c:c + 2048], in_=xq_g[(lo, 2)][:])
                c = xq_col(lo, 1536)
                eng.dma_start(out=xq_sb[:, c:c + 2048], in_=xq_g[(lo, 3)][:])

            # ---- helpers --------------------------------------------------
            ndum = [0]

            def warm(n):
                for _ in range(n):
                    dmy = PL.tile([128, QH], F32, tag="L",
                                  name=f"dmy{ndum[0]}")
                    ndum[0] += 1
                    nc.tensor.matmul(dmy[:, 0:SR], seed[:, 0:128],
                                     seed[:], start=True, stop=True)

            def q_range(r, wm=0):
                ps_q = PP.tile([128, SR], F32, tag="p", name=f"psq{r}")
                for k in range(8):
                    if k == 4 and wm:
                        warm(wm)
                    c = xq_col(k, r * SR)
                    nc.tensor.matmul(
                        ps_q[:], wd_s[:, k * 256:k * 256 + 128],
                        xq_sb[:, c:c + SR],
                        start=(k == 0), stop=(k == 7))
                nc.vector.tensor_scalar_add(qT2[:, r * SR:(r + 1) * SR],
                                            ps_q[:], bdq_s[:, 0:1])

            def kv_range(ri, wm=0):
                c0, w, par = KV_RANGES[ri]
                ps_kv = PP.tile([128, SR], F32, tag="p", name=f"pskv{ri}")
                for k in range(8):
                    if k == 4 and wm:
                        warm(wm)
                    if par == 0:
                        lhsT = wd_s[:, k * 256 + 128:k * 256 + 256]
                    else:
                        lhsT = wd_s[:, 2048 + k * 128:2048 + (k + 1) * 128]
                    c = xk_col(k, c0)
                    nc.tensor.matmul(
                        ps_kv[:, 0:w], lhsT, xk_sb[:, c:c + w],
                        start=(k == 0), stop=(k == 7))
                half, blk0 = _chunk_block(c0 // JC)
                kh = slice(0, 64) if half == 0 else slice(64, 128)
                vh = slice(64, 128) if half == 0 else slice(0, 64)
                nc.vector.tensor_scalar_add(
                    kT2[kh, blk0 * JC:blk0 * JC + w], ps_kv[kh, 0:w],
                    bdkv_s[kh, par:par + 1])
                nc.vector.tensor_scalar_add(
                    vTb[vh, c0:c0 + w], ps_kv[vh, 0:w],
                    bdkv_s[vh, par:par + 1])

            def v_trans(ri):
                c0, w, par = KV_RANGES[ri]
                vh = slice(64, 128) if par == 0 else slice(0, 64)
                idh = ident[64:128, :] if par == 0 else ident[0:64, :]
                nch = w // JC
                vt_ps = PT.tile([128, 256], F32R, tag="t", name=f"vt{ri}")
                for j in range(nch):
                    c = c0 // JC + j
                    nc.tensor.transpose(
                        vt_ps[:, j * 64:(j + 1) * 64],
                        vTb[vh, c * JC:(c + 1) * JC], idh)
                for j in range(nch):
                    c = c0 // JC + j
                    nc.vector.tensor_copy(v_aug[:, c * 65:c * 65 + 64],
                                          vt_ps[:, j * 64:(j + 1) * 64])

            # ================ main software pipeline =======================
            exs = {}
            ctx_tiles = {}
            nmm2 = [0]

            def mm1_exp(pas, ce, co):
                q0 = pas * QH
                lgs = []
                for c in (ce, co):
                    if c is None:
                        continue
                    half, blk = _chunk_block(c)
                    hs = slice(0, 64) if half == 0 else slice(64, 128)
                    lg = PL.tile([128, QH], F32, tag="L",
                                 name=f"lg{pas}_{c}")
                    for s2 in range(2):
                        nc.tensor.matmul(
                            lg[:, s2 * SR:(s2 + 1) * SR],
                            kT2[hs, blk * JC:(blk + 1) * JC],
                            qT2[hs, q0 + s2 * SR:q0 + (s2 + 1) * SR],
                            start=True, stop=True)
                    lgs.append((c, lg))
                for c, lg in lgs:
                    ex = ep.tile([128, QH], BF16, tag="e", name=f"ex{pas}_{c}")
                    nc.scalar.activation(ex[:], lg[:],
                                         mybir.ActivationFunctionType.Exp,
                                         bias=mb_s[:, c:c + 1], scale=SCALE)
                    exs[c] = ex

            def mm2(pas, c):
                ctx_ps = ctx_tiles[pas]
                i = nmm2[0]
                nmm2[0] += 1
                first = (i % NJK == 0)
                last = (i % NJK == NJK - 1)
                for s2 in range(2):
                    nc.tensor.matmul(
                        ctx_ps[:, s2 * SR:(s2 + 1) * SR],
                        v_aug[:, c * 65:(c + 1) * 65],
                        exs[c][:, s2 * SR:(s2 + 1) * SR],
                        start=first, stop=last)

            def ctx_evac(pas):
                q0 = pas * QH
                ctx_ps = ctx_tiles[pas]
                nc.vector.tensor_copy(ctxu[:, q0:q0 + QH], ctx_ps[0:65, :])
                nc.vector.tensor_copy(zr[64:65, q0:q0 + QH], ctx_ps[64:65, :])

            def z_recip(pas):
                # transpose Z [1,1024] -> query-major via 8 tiny f32r PE
                # transposes (K=2: row 65 is a discarded garbage column to
                # satisfy the fp32r ISA restriction), then one reciprocal
                q0 = pas * QH
                zt_ps = PT.tile([128, 16], F32R, tag="t", name=f"zt{pas}")
                for st in range(8):
                    nc.tensor.transpose(
                        zt_ps[:, 2 * st:2 * st + 2],
                        zr[64:66, q0 + st * JC:q0 + (st + 1) * JC],
                        idz[64:66, 0:2])
                nc.vector.reciprocal(rzbc[:, pas * 16:pas * 16 + 16],
                                     zt_ps[:, 0:16])

            def up_tile(st, tail=False):
                # out rows st*128:(st+1)*128 = (ctxu_st @ [Wu; bu]) * 1/Z_q
                osb = ob.tile([128, D], F16, tag="o", name=f"osb{st}")
                if tail:
                    up = PL.tile([128, QH], F32, tag="L", name=f"upt{st}")
                    ups = [up[:, 0:SR], up[:, SR:QH]]
                else:
                    ups = [PP.tile([128, SR], F32, tag="p", name=f"up{st}a"),
                           PT.tile([128, SR], F32, tag="t", name=f"up{st}b")]
                for s2 in range(2):
                    nc.tensor.matmul(
                        ups[s2], ctxu[:, st * JC:(st + 1) * JC],
                        wu_s[:, s2 * SR:(s2 + 1) * SR],
                        start=True, stop=True)
                for s2 in range(2):
                    src = ups[s2]
                    dst = osb[:, s2 * SR:(s2 + 1) * SR]
                    if tail and s2 == 0:
                        nc.scalar.mul(dst, src, rzbc[:, 2 * st:2 * st + 1])
                    else:
                        nc.vector.tensor_scalar_mul(dst, src,
                                                    rzbc[:, 2 * st:2 * st + 1])
                eng = nc.sync if (st < 8 or st % 2 == 0) else nc.scalar
                eng.dma_start(out=out_d[st * JC:(st + 1) * JC, :], in_=osb[:])

            # ---- prologue: ranges chase the DMAs, dummies bridge stalls ---
            warm(4)
            kv_range(0, wm=3)
            warm(2)
            q_range(0, wm=2)
            warm(2)
            q_range(1, wm=2)
            v_trans(0)
            warm(2)
            kv_range(1, wm=2)
            v_trans(1)
            warm(2)

            fillers = {
                (0, 3): [lambda: kv_range(2), lambda: v_trans(2)],
                (0, 4): [lambda: kv_range(3), lambda: v_trans(3)],
                (0, 5): [lambda: kv_range(4), lambda: v_trans(4)],
                (0, 6): [lambda: q_range(2)],
                (0, 7): [lambda: q_range(3)],
                (1, 2): [lambda: up_tile(0)],
                (1, 3): [lambda: up_tile(1)],
                (1, 4): [lambda: up_tile(2)],
                (1, 5): [lambda: up_tile(3)],
                (1, 6): [lambda: up_tile(4)],
                (1, 7): [lambda: up_tile(5)],
                (1, 8): [lambda: up_tile(6)],
            }

            for pas in range(2):
                steps = STEPS_A if pas == 0 else STEPS_B
                ctx_tiles[pas] = PC.tile([65, QH], F32, tag="c",
                                         name=f"ctx{pas}")
                for si, (ce, co) in enumerate(steps):
                    if si > 0:
                        pe, po = steps[si - 1]
                        mm2(pas, pe)
                        if po is not None:
                            mm2(pas, po)
                    elif pas == 1:
                        mm2(0, SOLO)
                        ctx_evac(0)
                    mm1_exp(pas, ce, co)
                    for f in fillers.get((pas, si), []):
                        f()
                    if pas == 1 and si == 1:
                        z_recip(0)
                if pas == 1:
                    mm2(1, SOLO)
            ctx_evac(1)
            z_recip(1)
            for st in range(7, 16):
                up_tile(st, tail=True)

    nc.compile()
    return nc


def get_graph():
    if "graph" not in _CACHE:
        _CACHE["graph"] = build_graph()
    return _CACHE["graph"]


def make_in_maps(x, attention_mask, Wd, bd, Wu, bu):
    # wd2: cols 0:2048 per-chunk [q|q|k|v], cols 2048:3072 per-chunk [v|k]
    wd2 = np.empty((128, 3072), np.float16)
    for k in range(8):
        blk = Wd[k * 128:(k + 1) * 128, :].astype(np.float16)
        q_, k_, v_ = blk[:, 0:64], blk[:, 64:128], blk[:, 128:192]
        wd2[:, k * 256:(k + 1) * 256] = np.concatenate([q_, q_, k_, v_], 1)
        wd2[:, 2048 + k * 128:2048 + (k + 1) * 128] = np.concatenate(
            [v_, k_], 1)
    bf16 = mybir.dt.np(mybir.dt.bfloat16)
    wu2 = np.ascontiguousarray(np.concatenate(
        [Wu, bu.reshape(1, D)], axis=0).astype(bf16))
    bdq2 = np.concatenate([bd[0:64], bd[0:64]]).reshape(128, 1).astype(np.float32)
    bdkv2 = np.stack([
        np.concatenate([bd[64:128], bd[128:192]]),
        np.concatenate([bd[128:192], bd[64:128]]),
    ], axis=1).astype(np.float32)
    per_batch = []
    for b in range(B):
        idx = np.nonzero(attention_mask[b])[0]
        n = len(idx)
        assert n <= K_CAP, f"unmasked key count {n} exceeds K_CAP={K_CAP}"
        idxp = np.concatenate([idx, np.zeros(K_CAP - n, np.int64)])
        xkT = x[b][idxp].T.astype(np.float16).reshape(
            8, 128, K_CAP).transpose(1, 0, 2)   # [128, slab, col]
        mb = np.full(K_CAP, MASKED_BIAS, np.float32)
        mb[:n] = LOGIT_SHIFT
        per_batch.append((xkT, np.ascontiguousarray(mb.reshape(NJK, 128).T)))
    in_maps = []
    for c in range(N_CORES):
        b, h = c // 2, c % 2
        xkT, mb = per_batch[b]
        xT = x[b, h * S_LOC:(h + 1) * S_LOC].T.astype(np.float16).reshape(
            8, 128, S_LOC).transpose(1, 0, 2)
        m = {
            "Wd2": wd2,
            "Wu2": wu2,
            "bd_q2": bdq2,
            "bd_kv2": bdkv2,
            "maskbias": mb,
        }
        for lo in (0, 4):
            hh = "lo" if lo == 0 else "hi"
            sl = slice(lo, lo + 4)
            m[f"xk_a_{hh}"] = np.ascontiguousarray(
                xkT[:, sl, 0:512]).reshape(128, -1)
            m[f"xk_b_{hh}"] = np.ascontiguousarray(
                xkT[:, sl, 512:1024]).reshape(128, -1)
            m[f"xk_c_{hh}"] = np.ascontiguousarray(
                xkT[:, sl, 1024:K_CAP]).reshape(128, -1)
            for r in range(4):
                m[f"xq_r{r}_{hh}"] = np.ascontiguousarray(
                    xT[:, sl, r * 512:(r + 1) * 512]).reshape(128, -1)
        in_maps.append(m)
    return in_maps


def kernel(x, attention_mask, Wd, bd, Wu, bu):
    from concourse import bass_utils

    x = np.asarray(x, dtype=np.float32)
    attention_mask = np.asarray(attention_mask)
    Wd = np.asarray(Wd, dtype=np.float32)
    bd = np.asarray(bd, dtype=np.float32)
    Wu = np.asarray(Wu, dtype=np.float32)
    bu = np.asarray(bu, dtype=np.float32)

    nc = get_graph()
    in_maps = make_in_maps(x, attention_mask, Wd, bd, Wu, bu)
    res = bass_utils.run_bass_kernel_spmd(nc, in_maps, list(range(N_CORES)))
    out = np.empty((B, S, D), dtype=np.float32)
    for c in range(N_CORES):
        b, h = c // 2, c % 2
        out[b, h * S_LOC:(h + 1) * S_LOC, :] = \
            res.results[c]["out"].astype(np.float32)
    return out


# revision 48
# speedup vs baseline: 1.2320x; 1.0331x over previous
"""Distributed Trainium2 (Bass/Tile) kernel for single-head latent attention.

Reference computation (B=4, S=4096, D=1024, DL=64):
    qkv = x @ Wd + bd; q,k,v = split(qkv)
    logits = (q @ k^T) / sqrt(DL) / TEMP, key-masked
    out = softmax(logits) @ v @ Wu + bu

Sharding: data-parallel over (batch, seq-half) -> 8 shards of 2048 query
rows; each core recomputes K/V for its batch's compacted keys (no
collectives).

Design notes (vs the 124us baseline):
  - Host-side mask compaction (only ~2048 unmasked keys kept, cap 2176).
  - Inputs are pre-split host-side into C-contiguous per-transfer blocks and
    the SBUF side is laid out identically, so both ends of every DMA are
    4KB+/partition contiguous -> big packets -> full HW-queue bandwidth.
    Only sync + scalar have hardware DGE queues; gpsimd's is ~10x slower
    (software DGE) and carries only tiny consts.
  - Projections chase the arriving range blocks; attention starts while
    later ranges are still in flight.  Dummy matmuls bridge every DMA stall
    so the PE HAM clock-gate stays at 2.4 GHz.
  - MM1 row-tiled (contraction DL=64): chunk pairs run concurrently on the
    two 64-row halves of the PE array.  kT has a parity layout (even
    kv-ranges' k in partitions 0:64, odd in 64:128) produced with
    per-range-swapped [k|v]/[v|k] projection weights so no cross-partition
    moves are needed; q is duplicated into both halves for free by
    duplicating Wd's q columns (M=64 -> M=128 projection).
  - Postponed normalization: out = (ctxU @ [Wu; bu*Z]) * (1/Z) per query.
    MM2's stationary is [v | ones], so ctx rows 0:64 are unnormalized ctx
    and row 64 is Z; the up-projection contracts over 65 rows (row 64 = bu)
    and the PSUM evacuation is a bias-free per-partition scale by 1/Z that
    either the vector OR scalar engine can apply (tail splits across both).
    ctxU flows bf16 (values span e^+-55: overflows fp16, not bf16).
  - Z is transposed to query-major via 8 tiny f32r PE transposes per pass +
    one reciprocal; scalar engine otherwise does exp ONLY (34 [128,1024]
    ACTIVATEs ~ 38us is the pacing floor).
"""

import sys

if "/opt/trn_rl_repo" not in sys.path:
    sys.path.insert(0, "/opt/trn_rl_repo")

import numpy as np

from concourse import bacc, tile
from concourse import mybir
from concourse.masks import make_identity

F32 = mybir.dt.float32
F32R = mybir.dt.float32r
BF16 = mybir.dt.bfloat16
F16 = mybir.dt.float16

B, S, D, DL = 4, 4096, 1024, 64
N_CORES = 8
S_LOC = S // 2          # 2048 query rows per core
SR = 512
JC = 128                # key chunk
NJK = 17                # compacted key chunks
K_CAP = NJK * JC        # 2176 >= max unmasked keys per batch
QH = 1024               # one attention pass = 1024 query columns
SCALE = 1.25            # 1/sqrt(64)/0.1
LOGIT_SHIFT = -40.0
MASKED_BIAS = -1e30

# kv ranges: (col0, width, parity).  Even ranges project with [k|v] weights
# (k -> psum rows 0:64), odd with [v|k] (k -> rows 64:128), so the k-half
# always evacuates same-partition into its kT2 half.
KV_RANGES = [(0, 512, 0), (512, 512, 1), (1024, 512, 0), (1536, 512, 1),
             (2048, 128, 0)]
# chunk -> kT2 half/block: top (rows 0:64) = ranges 0,2,4; bottom = 1,3
TOP_CHUNKS = [0, 1, 2, 3, 8, 9, 10, 11, 16]
BOT_CHUNKS = [4, 5, 6, 7, 12, 13, 14, 15]
# pass A: step order follows DMA arrival (range 0 chunks first); pass B has
# all data resident -> strict top/bot pairs for full MM1 concurrency
STEPS_A = [(0, 1), (2, 4), (3, 5), (6, 7), (8, 9), (10, 12), (11, 13),
           (14, 15), (16, None)]
STEPS_B = [(0, 4), (1, 5), (2, 6), (3, 7), (8, 12), (9, 13), (10, 14),
           (11, 15), (16, None)]
SOLO = 16

_CACHE = {}


def _chunk_block(c):
    """kT2 (half, block) for chunk c."""
    if c in TOP_CHUNKS:
        return 0, TOP_CHUNKS.index(c)
    return 1, BOT_CHUNKS.index(c)


def build_graph():
    nc = bacc.Bacc("TRN2", target_bir_lowering=False, debug=False,
                   num_devices=N_CORES)

    # Wd packed as: cols 0:2048 = per-chunk [q|q|k|v] (256 each, the only
    # part needed before attention starts), cols 2048:3072 = per-chunk [v|k]
    wd_d = nc.dram_tensor("Wd2", [128, 3072], F16, kind="ExternalInput").ap()
    wu_d = nc.dram_tensor("Wu2", [65, D], BF16, kind="ExternalInput").ap()
    bdq_d = nc.dram_tensor("bd_q2", [128, 1], F32, kind="ExternalInput").ap()
    bdkv_d = nc.dram_tensor("bd_kv2", [128, 2], F32, kind="ExternalInput").ap()
    mb_d = nc.dram_tensor("maskbias", [128, NJK], F32, kind="ExternalInput").ap()
    out_d = nc.dram_tensor("out", [S_LOC, D], F16, kind="ExternalOutput").ap()

    xk_g, xq_g = {}, {}
    for lo in (0, 4):
        h = "lo" if lo == 0 else "hi"
        xk_g[(lo, 0)] = nc.dram_tensor(
            f"xk_a_{h}", [128, 4 * 512], F16, kind="ExternalInput").ap()
        xk_g[(lo, 512)] = nc.dram_tensor(
            f"xk_b_{h}", [128, 4 * 512], F16, kind="ExternalInput").ap()
        xk_g[(lo, 1024)] = nc.dram_tensor(
            f"xk_c2_{h}", [128, 4 * 512], F16, kind="ExternalInput").ap()
        xk_g[(lo, 1536)] = nc.dram_tensor(
            f"xk_c34_{h}", [128, 4 * (K_CAP - 1536)], F16,
            kind="ExternalInput").ap()
        for r in range(4):
            xq_g[(lo, r)] = nc.dram_tensor(
                f"xq_r{r}_{h}", [128, 4 * 512], F16,
                kind="ExternalInput").ap()

    def xk_col(k, c):
        # flat xk_sb column for slab k, key-col c (group-blocked layout)
        half, kl = k // 4, k % 4
        if c < 512:
            return half * 2048 + kl * 512 + c
        if c < 1024:
            return 4096 + half * 2048 + kl * 512 + (c - 512)
        if c < 1536:
            return 8192 + half * 2048 + kl * 512 + (c - 1024)
        return 12288 + half * 2560 + kl * 640 + (c - 1536)

    def xq_col(k, c):
        half, kl = k // 4, k % 4
        return (c // 512) * 4096 + half * 2048 + kl * 512 + (c % 512)

    with tile.TileContext(nc) as tc, nc.allow_low_precision(
            reason="bf16/f16 tiles feed full-rate PE matmuls; ~10-bit "
                   "mantissas are far inside the 2e-2 error budget"):
        with (
            tc.tile_pool(name="consts", bufs=1) as consts,
            tc.tile_pool(name="acts", bufs=1) as acts,
            tc.tile_pool(name="ep", bufs=4) as ep,
            tc.tile_pool(name="ob", bufs=3) as ob,
            tc.tile_pool(name="PL", bufs=2, space="PSUM") as PL,
            tc.tile_pool(name="PC", bufs=1, space="PSUM") as PC,
            tc.tile_pool(name="PP", bufs=1, space="PSUM") as PP,
            tc.tile_pool(name="PT", bufs=1, space="PSUM") as PT,
        ):
            # ---- warm-up seeds, all on the scalar engine (its preamble is
            # the shortest) so PE dummies + ACT table load start ~1us ------
            seed = consts.tile([128, SR], F16)
            nc.scalar.memzero(seed[:])
            actwarm = consts.tile([128, 32], F32)
            nc.scalar.activation(actwarm[:], seed[:, 0:32],
                                 mybir.ActivationFunctionType.Exp)

            # ---- small consts ---------------------------------------------
            identf = consts.tile([128, 64], F32)
            nc.vector.memset(identf[:], 0.0)
            make_identity(nc, identf[0:64, :], nomemset=True)
            make_identity(nc, identf[64:128, :], nomemset=True)
            ident = consts.tile([128, 64], F32R)
            nc.vector.tensor_copy(ident[:], identf[:])
            idzf = consts.tile([128, 4], F32)
            nc.vector.memset(idzf[:], 0.0)
            make_identity(nc, idzf[64:66, 0:2], nomemset=True)
            idz = consts.tile([128, 4], F32R)
            nc.vector.tensor_copy(idz[:], idzf[:])

            # ---- DMA'd consts (gpsimd slow queue: tiny / late-needed) -----
            bdq_s = consts.tile([128, 1], F32)
            nc.gpsimd.dma_start(out=bdq_s[:], in_=bdq_d[:])
            bdkv_s = consts.tile([128, 2], F32)
            nc.gpsimd.dma_start(out=bdkv_s[:], in_=bdkv_d[:])
            mb_s = consts.tile([128, NJK], F32)
            nc.gpsimd.dma_start(out=mb_s[:], in_=mb_d[:])
            wu_s = consts.tile([65, D], BF16)
            nc.gpsimd.dma_start(out=wu_s[:], in_=wu_d[:])
            wd_s = consts.tile([128, 3072], F16)

            # ---- activation tiles (x slabs group-blocked, see xk_col) -----
            xq_sb = acts.tile([128, 8 * S_LOC], F16)
            xk_sb = acts.tile([128, 8 * K_CAP], F16)
            qT2 = acts.tile([128, S_LOC], F16)       # q in both halves
            kT2 = acts.tile([128, 9 * JC], F16)      # parity layout
            vTb = acts.tile([128, K_CAP], F32R)      # v staging (half by rng)
            v_aug = acts.tile([128, NJK * 65], BF16)  # [v(64)|ones] per chunk
            nc.vector.memset(v_aug[:], 1.0)
            ctxu = acts.tile([65, S_LOC], BF16)      # rows 0:64 ctx, 64 = Z
            zr = acts.tile([128, S_LOC], F32R)       # Z row staging (row 64)
            rzbc = acts.tile([128, 32], F32)         # 1/Z query-major, 2*st

            # ---- input DMAs: contiguous blocks, two fast queues -----------
            for lo, eng in ((0, nc.sync), (4, nc.scalar)):
                eng.dma_start(out=wd_s[:, lo * 256:(lo + 4) * 256],
                              in_=wd_d[:, lo * 256:(lo + 4) * 256])
                c = xk_col(lo, 0)
                eng.dma_start(out=xk_sb[:, c:c + 2048], in_=xk_g[(lo, 0)][:])
                c = xq_col(lo, 0)
                eng.dma_start(out=xq_sb[:, c:c + 2048], in_=xq_g[(lo, 0)][:])
                c = xq_col(lo, 512)
                eng.dma_start(out=xq_sb[:, c:c + 2048], in_=xq_g[(lo, 1)][:])
                # odd-range [v|k] weight block
                eng.dma_start(out=wd_s[:, 2048 + lo * 128:2048 + (lo + 4) * 128],
                              in_=wd_d[:, 2048 + lo * 128:2048 + (lo + 4) * 128])
                c = xk_col(lo, 512)
                eng.dma_start(out=xk_sb[:, c:c + 2048], in_=xk_g[(lo, 512)][:])
                c = xk_col(lo, 1024)
                eng.dma_start(out=xk_sb[:, c:c + 2048],
                              in_=xk_g[(lo, 1024)][:])
                c = xq_col(lo, 1024)
                eng.dma_start(out=xq_sb[:, c:c + 2048], in_=xq_g[(lo, 2)][:])
                c = xk_col(lo, 1536)
                eng.dma_start(out=xk_sb[:, c:c + 2560],
                              in_=xk_g[(lo, 1536)][:])
                c = xq_col(lo, 1536)
                eng.dma_start(out=xq_sb[:, c:c + 2048], in_=xq_g[(lo, 3)][:])

            # ---- helpers --------------------------------------------------
            ndum = [0]

            def warm(n):
                for _ in range(n):
                    dmy = PL.tile([128, QH], F32, tag="L",
                                  name=f"dmy{ndum[0]}")
                    ndum[0] += 1
                    nc.tensor.matmul(dmy[:, 0:SR], seed[:, 0:128],
                                     seed[:], start=True, stop=True)

            def q_range(r, wm=0):
                ps_q = PP.tile([128, SR], F32, tag="p", name=f"psq{r}")
                for k in range(8):
                    if k == 4 and wm:
                        warm(wm)
                    c = xq_col(k, r * SR)
                    nc.tensor.matmul(
                        ps_q[:], wd_s[:, k * 256:k * 256 + 128],
                        xq_sb[:, c:c + SR],
                        start=(k == 0), stop=(k == 7))
                nc.vector.tensor_scalar_add(qT2[:, r * SR:(r + 1) * SR],
                                            ps_q[:], bdq_s[:, 0:1])

            def kv_range(ri, wm=0):
                c0, w, par = KV_RANGES[ri]
                ps_kv = PP.tile([128, SR], F32, tag="p", name=f"pskv{ri}")
                for k in range(8):
                    if k == 4 and wm:
                        warm(wm)
                    if par == 0:
                        lhsT = wd_s[:, k * 256 + 128:k * 256 + 256]
                    else:
                        lhsT = wd_s[:, 2048 + k * 128:2048 + (k + 1) * 128]
                    c = xk_col(k, c0)
                    nc.tensor.matmul(
                        ps_kv[:, 0:w], lhsT, xk_sb[:, c:c + w],
                        start=(k == 0), stop=(k == 7))
                half, blk0 = _chunk_block(c0 // JC)
                kh = slice(0, 64) if half == 0 else slice(64, 128)
                vh = slice(64, 128) if half == 0 else slice(0, 64)
                nc.vector.tensor_scalar_add(
                    kT2[kh, blk0 * JC:blk0 * JC + w], ps_kv[kh, 0:w],
                    bdkv_s[kh, par:par + 1])
                nc.vector.tensor_scalar_add(
                    vTb[vh, c0:c0 + w], ps_kv[vh, 0:w],
                    bdkv_s[vh, par:par + 1])

            def v_trans(ri):
                c0, w, par = KV_RANGES[ri]
                vh = slice(64, 128) if par == 0 else slice(0, 64)
                idh = ident[64:128, :] if par == 0 else ident[0:64, :]
                nch = w // JC
                vt_ps = PT.tile([128, 256], F32R, tag="t", name=f"vt{ri}")
                for j in range(nch):
                    c = c0 // JC + j
                    nc.tensor.transpose(
                        vt_ps[:, j * 64:(j + 1) * 64],
                        vTb[vh, c * JC:(c + 1) * JC], idh)
                for j in range(nch):
                    c = c0 // JC + j
                    nc.vector.tensor_copy(v_aug[:, c * 65:c * 65 + 64],
                                          vt_ps[:, j * 64:(j + 1) * 64])

            # ================ main software pipeline =======================
            exs = {}
            ctx_tiles = {}
            nmm2 = [0]

            def mm1_exp(pas, ce, co):
                q0 = pas * QH
                lgs = []
                for c in (ce, co):
                    if c is None:
                        continue
                    half, blk = _chunk_block(c)
                    hs = slice(0, 64) if half == 0 else slice(64, 128)
                    lg = PL.tile([128, QH], F32, tag="L",
                                 name=f"lg{pas}_{c}")
                    for s2 in range(2):
                        nc.tensor.matmul(
                            lg[:, s2 * SR:(s2 + 1) * SR],
                            kT2[hs, blk * JC:(blk + 1) * JC],
                            qT2[hs, q0 + s2 * SR:q0 + (s2 + 1) * SR],
                            start=True, stop=True)
                    lgs.append((c, lg))
                for c, lg in lgs:
                    ex = ep.tile([128, QH], BF16, tag="e", name=f"ex{pas}_{c}")
                    nc.scalar.activation(ex[:], lg[:],
                                         mybir.ActivationFunctionType.Exp,
                                         bias=mb_s[:, c:c + 1], scale=SCALE)
                    exs[c] = ex

            def mm2(pas, c):
                ctx_ps = ctx_tiles[pas]
                i = nmm2[0]
                nmm2[0] += 1
                first = (i % NJK == 0)
                last = (i % NJK == NJK - 1)
                for s2 in range(2):
                    nc.tensor.matmul(
                        ctx_ps[:, s2 * SR:(s2 + 1) * SR],
                        v_aug[:, c * 65:(c + 1) * 65],
                        exs[c][:, s2 * SR:(s2 + 1) * SR],
                        start=first, stop=last)

            def ctx_evac(pas):
                q0 = pas * QH
                ctx_ps = ctx_tiles[pas]
                nc.vector.tensor_copy(ctxu[:, q0:q0 + QH], ctx_ps[0:65, :])
                nc.vector.tensor_copy(zr[64:65, q0:q0 + QH], ctx_ps[64:65, :])

            def z_recip(pas):
                # transpose Z [1,1024] -> query-major via 8 tiny f32r PE
                # transposes (K=2: row 65 is a discarded garbage column to
                # satisfy the fp32r ISA restriction), then one reciprocal
                q0 = pas * QH
                zt_ps = PT.tile([128, 16], F32R, tag="t", name=f"zt{pas}")
                for st in range(8):
                    nc.tensor.transpose(
                        zt_ps[:, 2 * st:2 * st + 2],
                        zr[64:66, q0 + st * JC:q0 + (st + 1) * JC],
                        idz[64:66, 0:2])
                nc.vector.reciprocal(rzbc[:, pas * 16:pas * 16 + 16],
                                     zt_ps[:, 0:16])

            def up_tile(st, tail=False):
                # out rows st*128:(st+1)*128 = (ctxu_st @ [Wu; bu]) * 1/Z_q
                osb = ob.tile([128, D], F16, tag="o", name=f"osb{st}")
                if tail:
                    up = PL.tile([128, QH], F32, tag="L", name=f"upt{st}")
                    ups = [up[:, 0:SR], up[:, SR:QH]]
                else:
                    ups = [PP.tile([128, SR], F32, tag="p", name=f"up{st}a"),
                           PT.tile([128, SR], F32, tag="t", name=f"up{st}b")]
                for s2 in range(2):
                    nc.tensor.matmul(
                        ups[s2], ctxu[:, st * JC:(st + 1) * JC],
                        wu_s[:, s2 * SR:(s2 + 1) * SR],
                        start=True, stop=True)
                for s2 in range(2):
                    src = ups[s2]
                    dst = osb[:, s2 * SR:(s2 + 1) * SR]
                    if tail and s2 == 0:
                        nc.scalar.mul(dst, src, rzbc[:, 2 * st:2 * st + 1])
                    else:
                        nc.vector.tensor_scalar_mul(dst, src,
                                                    rzbc[:, 2 * st:2 * st + 1])
                eng = nc.sync if (st < 8 or st % 2 == 0) else nc.scalar
                eng.dma_start(out=out_d[st * JC:(st + 1) * JC, :], in_=osb[:])

            # ---- prologue: ranges chase the DMAs, dummies bridge stalls ---
            warm(4)
            kv_range(0, wm=3)
            warm(2)
            q_range(0, wm=2)
            warm(2)
            q_range(1, wm=2)
            v_trans(0)
            warm(2)

            fillers = {
                (0, 0): [lambda: kv_range(1, wm=2), lambda: v_trans(1)],
                (0, 2): [lambda: kv_range(2), lambda: v_trans(2)],
                (0, 4): [lambda: kv_range(3), lambda: v_trans(3)],
                (0, 5): [lambda: kv_range(4), lambda: v_trans(4)],
                (0, 6): [lambda: q_range(2)],
                (0, 7): [lambda: q_range(3)],
                (1, 2): [lambda: up_tile(0)],
                (1, 3): [lambda: up_tile(1)],
                (1, 4): [lambda: up_tile(2)],
                (1, 5): [lambda: up_tile(3)],
                (1, 6): [lambda: up_tile(4)],
                (1, 7): [lambda: up_tile(5)],
                (1, 8): [lambda: up_tile(6)],
            }

            for pas in range(2):
                steps = STEPS_A if pas == 0 else STEPS_B
                ctx_tiles[pas] = PC.tile([65, QH], F32, tag="c",
                                         name=f"ctx{pas}")
                for si, (ce, co) in enumerate(steps):
                    if si > 0:
                        pe, po = steps[si - 1]
                        mm2(pas, pe)
                        if po is not None:
                            mm2(pas, po)
                    elif pas == 1:
                        mm2(0, SOLO)
                        ctx_evac(0)
                    mm1_exp(pas, ce, co)
                    for f in fillers.get((pas, si), []):
                        f()
                    if pas == 1 and si == 1:
                        z_recip(0)
                if pas == 1:
                    mm2(1, SOLO)
            ctx_evac(1)
            z_recip(1)
            for st in range(7, 16):
                up_tile(st, tail=True)

    nc.compile()
    return nc


def get_graph():
    if "graph" not in _CACHE:
        _CACHE["graph"] = build_graph()
    return _CACHE["graph"]


def make_in_maps(x, attention_mask, Wd, bd, Wu, bu):
    # wd2: cols 0:2048 per-chunk [q|q|k|v], cols 2048:3072 per-chunk [v|k]
    wd2 = np.empty((128, 3072), np.float16)
    for k in range(8):
        blk = Wd[k * 128:(k + 1) * 128, :].astype(np.float16)
        q_, k_, v_ = blk[:, 0:64], blk[:, 64:128], blk[:, 128:192]
        wd2[:, k * 256:(k + 1) * 256] = np.concatenate([q_, q_, k_, v_], 1)
        wd2[:, 2048 + k * 128:2048 + (k + 1) * 128] = np.concatenate(
            [v_, k_], 1)
    bf16 = mybir.dt.np(mybir.dt.bfloat16)
    wu2 = np.ascontiguousarray(np.concatenate(
        [Wu, bu.reshape(1, D)], axis=0).astype(bf16))
    bdq2 = np.concatenate([bd[0:64], bd[0:64]]).reshape(128, 1).astype(np.float32)
    bdkv2 = np.stack([
        np.concatenate([bd[64:128], bd[128:192]]),
        np.concatenate([bd[128:192], bd[64:128]]),
    ], axis=1).astype(np.float32)
    per_batch = []
    for b in range(B):
        idx = np.nonzero(attention_mask[b])[0]
        n = len(idx)
        assert n <= K_CAP, f"unmasked key count {n} exceeds K_CAP={K_CAP}"
        idxp = np.concatenate([idx, np.zeros(K_CAP - n, np.int64)])
        xkT = x[b][idxp].T.astype(np.float16).reshape(
            8, 128, K_CAP).transpose(1, 0, 2)   # [128, slab, col]
        mb = np.full(K_CAP, MASKED_BIAS, np.float32)
        mb[:n] = LOGIT_SHIFT
        per_batch.append((xkT, np.ascontiguousarray(mb.reshape(NJK, 128).T)))
    in_maps = []
    for c in range(N_CORES):
        b, h = c // 2, c % 2
        xkT, mb = per_batch[b]
        xT = x[b, h * S_LOC:(h + 1) * S_LOC].T.astype(np.float16).reshape(
            8, 128, S_LOC).transpose(1, 0, 2)
        m = {
            "Wd2": wd2,
            "Wu2": wu2,
            "bd_q2": bdq2,
            "bd_kv2": bdkv2,
            "maskbias": mb,
        }
        for lo in (0, 4):
            hh = "lo" if lo == 0 else "hi"
            sl = slice(lo, lo + 4)
            m[f"xk_a_{hh}"] = np.ascontiguousarray(
                xkT[:, sl, 0:512]).reshape(128, -1)
            m[f"xk_b_{hh}"] = np.ascontiguousarray(
                xkT[:, sl, 512:1024]).reshape(128, -1)
            m[f"xk_c2_{hh}"] = np.ascontiguousarray(
                xkT[:, sl, 1024:1536]).reshape(128, -1)
            m[f"xk_c34_{hh}"] = np.ascontiguousarray(
                xkT[:, sl, 1536:K_CAP]).reshape(128, -1)
            for r in range(4):
                m[f"xq_r{r}_{hh}"] = np.ascontiguousarray(
                    xT[:, sl, r * 512:(r + 1) * 512]).reshape(128, -1)
        in_maps.append(m)
    return in_maps


def kernel(x, attention_mask, Wd, bd, Wu, bu):
    from concourse import bass_utils

    x = np.asarray(x, dtype=np.float32)
    attention_mask = np.asarray(attention_mask)
    Wd = np.asarray(Wd, dtype=np.float32)
    bd = np.asarray(bd, dtype=np.float32)
    Wu = np.asarray(Wu, dtype=np.float32)
    bu = np.asarray(bu, dtype=np.float32)

    nc = get_graph()
    in_maps = make_in_maps(x, attention_mask, Wd, bd, Wu, bu)
    res = bass_utils.run_bass_kernel_spmd(nc, in_maps, list(range(N_CORES)))
    out = np.empty((B, S, D), dtype=np.float32)
    for c in range(N_CORES):
        b, h = c // 2, c % 2
        out[b, h * S_LOC:(h + 1) * S_LOC, :] = \
            res.results[c]["out"].astype(np.float32)
    return out
